# revision 1
# baseline (speedup 1.0000x reference)
"""Trainium2 Bass kernel for nn_Criterion_32830730011569.

Strategy: 8 cores = (image b in 0..3) x (H-half h in 0..1). Each core streams
its [96,192] pixel slice of the big tensors from HBM once:
  - dice: per-pixel softmax over matched portion channels is reformulated so no
    channel gather is needed in the hot loop:
      num_b = 2 * sum_m C[me[m], mq[m]],  C = sum_pixels (true/Z)^T (exp(por)*mask)
    accumulated as bf16 matmuls into one [96,160] PSUM tile; den_b = sum(true) +
    H*W (softmax sums to 1) with the constant added on host.
  - occupancy CE: streamed logsumexp + label-select.
  - 7x7-window BCE: indirect DMAs gather window rows (one offset/partition);
    each half-core sums only the window pixels that live in its slice.
  - class / NLL: tiny one-hot matmul gathers.
Each core returns 7 partial sums; the host combines them into the scalar loss.
"""
import sys

sys.path.insert(0, "/opt/trn_rl_repo")
import numpy as np

B, H, W, Q, E, M, K, WIN = 4, 192, 192, 160, 96, 96, 4, 7
NO_E = 0.1
HALF = H // 2          # rows per core slice
NPIX = HALF * W        # 18432 pixels per slice
P = 128                # partitions
J = NPIX // P          # 144 pixels per partition (p-major)
NCHUNK = 8
JC = J // NCHUNK       # 18
MAGIC = 8388608.0      # 2^23
MAGIC_I = 0x4B000000

_CACHE = {}


def _build_nc():
    import os
    import concourse.bass as bass
    import concourse.bacc as bacc
    import concourse.tile as tile
    from concourse import mybir

    DIS = set(os.environ.get("KDIS", "").split(","))

    f32 = mybir.dt.float32
    i32 = mybir.dt.int32
    bf16 = mybir.dt.bfloat16
    AF = mybir.ActivationFunctionType
    OP = mybir.AluOpType
    AX = mybir.AxisListType

    nc = bacc.Bacc("TRN2", target_bir_lowering=False, debug=False, num_devices=8)

    # ---- external I/O ----
    true_sl = nc.dram_tensor("true_sl", [NPIX, E], f32, kind="ExternalInput")
    por_sl = nc.dram_tensor("por_sl", [NPIX, Q], f32, kind="ExternalInput")
    bin_sl = nc.dram_tensor("bin_sl", [NPIX, Q], f32, kind="ExternalInput")
    occ_sl = nc.dram_tensor("occ_sl", [NPIX, K], f32, kind="ExternalInput")
    occt_f = nc.dram_tensor("occt_f", [P, J], f32, kind="ExternalInput")
    me_colf = nc.dram_tensor("me_colf", [M, 1], f32, kind="ExternalInput")
    mq_colf = nc.dram_tensor("mq_colf", [M, 1], f32, kind="ExternalInput")
    me_row_rep = nc.dram_tensor("me_row_rep", [E, M], f32, kind="ExternalInput")
    mq_row_rep_a = nc.dram_tensor("mq_row_rep_a", [P, M], f32, kind="ExternalInput")
    mq_row_rep_b = nc.dram_tensor("mq_row_rep_b", [Q - P, M], f32, kind="ExternalInput")
    iota_q_row = nc.dram_tensor("iota_q_row", [M, Q], f32, kind="ExternalInput")
    iota_e_row = nc.dram_tensor("iota_e_row", [M, E], f32, kind="ExternalInput")
    iota_p_a = nc.dram_tensor("iota_p_a", [P, 1], f32, kind="ExternalInput")
    iota_p_b = nc.dram_tensor("iota_p_b", [Q - P, 1], f32, kind="ExternalInput")
    iota_p_96 = nc.dram_tensor("iota_p_96", [E, 1], f32, kind="ExternalInput")
    drof_rep = nc.dram_tensor("drof_rep", [M, WIN], f32, kind="ExternalInput")
    inc_pts = nc.dram_tensor("inc_pts", [E, 2], f32, kind="ExternalInput")
    pos_a = nc.dram_tensor("pos_a", [P, 2], f32, kind="ExternalInput")
    pos_b = nc.dram_tensor("pos_b", [Q - P, 2], f32, kind="ExternalInput")
    chol_a = nc.dram_tensor("chol_a", [P, 4], f32, kind="ExternalInput")
    chol_b = nc.dram_tensor("chol_b", [Q - P, 4], f32, kind="ExternalInput")
    iel_row = nc.dram_tensor("iel_row", [1, Q], f32, kind="ExternalInput")
    rb_neg = nc.dram_tensor("rb_neg", [M, 1], f32, kind="ExternalInput")
    partials = nc.dram_tensor("partials", [1, 8], f32, kind="ExternalOutput")

    def bc(ap, pos, count):
        """Insert a stride-0 broadcast dim into an AP at free-dim position pos."""
        new = list(ap.ap)
        new.insert(pos, [0, count])
        return bass.AP(tensor=ap.tensor, offset=ap.offset, ap=new)

    from contextlib import ExitStack

    with tile.TileContext(nc) as tc, ExitStack() as ctx:
        sing = ctx.enter_context(tc.tile_pool(name="sing", bufs=1))
        big = ctx.enter_context(tc.tile_pool(name="big", bufs=2))
        ps = ctx.enter_context(tc.tile_pool(name="ps", bufs=1, space="PSUM"))

        # ---------- small loads ----------
        def load(dram, shape):
            nm = dram.name + "_sb"
            t = sing.tile(shape, f32, name=nm, tag=nm)
            nc.sync.dma_start(out=t[:], in_=dram.ap())
            return t

        me_c = load(me_colf, [M, 1])
        mq_c = load(mq_colf, [M, 1])
        me_rr = load(me_row_rep, [E, M])
        mq_rra = load(mq_row_rep_a, [P, M])
        mq_rrb = load(mq_row_rep_b, [Q - P, M])
        io_q = load(iota_q_row, [M, Q])
        io_e = load(iota_e_row, [M, E])
        io_pa = load(iota_p_a, [P, 1])
        io_pb = load(iota_p_b, [Q - P, 1])
        io_p96 = load(iota_p_96, [E, 1])
        drof = load(drof_rep, [M, WIN])
        inc_sb = load(inc_pts, [E, 2])
        posa = load(pos_a, [P, 2])
        posb = load(pos_b, [Q - P, 2])
        chola = load(chol_a, [P, 4])
        cholb = load(chol_b, [Q - P, 4])
        iel = load(iel_row, [1, Q])
        rbn = load(rb_neg, [M, 1])

        def emit_ln(pref, out, x, pp, ff):
            """out = ln(x) for positive normal floats.

            Bit-extract exponent/mantissa (no float<->int casts needed), 3-term
            series on the reduced mantissa, then 2 Newton steps y += x*e^-y - 1
            using the ACT Exp table.
            """
            LN2 = 0.6931471805599453
            SQRT2 = 1.4142135623730951

            def T(nm, dt=f32):
                return sing.tile([pp, ff], dt, name=f"{pref}_{nm}", tag=f"{pref}_{nm}")

            xb = x.bitcast(i32)
            ei = T("ei", i32)
            nc.vector.tensor_scalar(out=ei[:], in0=xb, scalar1=23, scalar2=MAGIC_I,
                                    op0=OP.arith_shift_right, op1=OP.bitwise_or)
            ef = T("ef")
            nc.vector.tensor_scalar(out=ef[:], in0=ei[:].bitcast(f32),
                                    scalar1=-(MAGIC + 127.0), scalar2=None, op0=OP.add)
            mi = T("mi", i32)
            nc.vector.tensor_scalar(out=mi[:], in0=xb, scalar1=0x007FFFFF,
                                    scalar2=0x3F800000, op0=OP.bitwise_and, op1=OP.bitwise_or)
            mf = mi[:].bitcast(f32)
            cf = T("cf")
            nc.vector.tensor_scalar(out=cf[:], in0=mf, scalar1=SQRT2, scalar2=None, op0=OP.is_ge)
            hf = T("hf")
            nc.vector.tensor_scalar(out=hf[:], in0=cf[:], scalar1=-0.5, scalar2=1.0,
                                    op0=OP.mult, op1=OP.add)
            u = T("u")
            nc.vector.tensor_tensor(out=u[:], in0=mf, in1=hf[:], op=OP.mult)
            nc.vector.tensor_tensor(out=ef[:], in0=ef[:], in1=cf[:], op=OP.add)
            nc.vector.tensor_scalar(out=u[:], in0=u[:], scalar1=-1.0, scalar2=None, op0=OP.add)
            v = T("v")
            nc.vector.tensor_scalar(out=v[:], in0=u[:], scalar1=-0.25, scalar2=1.0 / 3.0,
                                    op0=OP.mult, op1=OP.add)
            nc.vector.tensor_tensor(out=v[:], in0=v[:], in1=u[:], op=OP.mult)
            nc.vector.tensor_scalar(out=v[:], in0=v[:], scalar1=-0.5, scalar2=None, op0=OP.add)
            nc.vector.tensor_tensor(out=v[:], in0=v[:], in1=u[:], op=OP.mult)
            nc.vector.tensor_scalar(out=v[:], in0=v[:], scalar1=1.0, scalar2=None, op0=OP.add)
            nc.vector.tensor_tensor(out=v[:], in0=v[:], in1=u[:], op=OP.mult)
            y = out
            nc.vector.tensor_scalar(out=y, in0=ef[:], scalar1=LN2, scalar2=None, op0=OP.mult)
            nc.vector.tensor_tensor(out=y, in0=y, in1=v[:], op=OP.add)
            ey = T("ey")
            w = T("w")
            for _ in range(2):
                nc.scalar.activation(out=ey[:], in_=y, func=AF.Exp, scale=-1.0)
                nc.vector.tensor_tensor(out=w[:], in0=ey[:], in1=x, op=OP.mult)
                nc.vector.tensor_scalar(out=w[:], in0=w[:], scalar1=-1.0, scalar2=None, op0=OP.add)
                nc.vector.tensor_tensor(out=y, in0=y, in1=w[:], op=OP.add)

        def emit_softplus(pref, out, x, pp, ff):
            """out = ln(1 + exp(x)) (inputs are O(1) logits, no overflow)."""
            opx = sing.tile([pp, ff], f32, name=pref + "_opx", tag=pref + "_opx")
            nc.scalar.activation(out=opx[:], in_=x, func=AF.Exp)
            nc.vector.tensor_scalar(out=opx[:], in0=opx[:], scalar1=1.0, scalar2=None, op0=OP.add)
            emit_ln(pref, out, opx[:], pp, ff)

        ones = sing.tile([P, 1], f32)
        nc.vector.memset(ones[:], 1.0)
        onesw = sing.tile([E, P], f32)
        nc.vector.memset(onesw[:], 1.0)

        stats = sing.tile([P, 6], f32)
        nc.vector.memset(stats[:], 0.0)
        res = sing.tile([1, 8], f32)
        nc.vector.memset(res[:], 0.0)

        # ---------- one-hot selectors ----------
        Mq = sing.tile([M, Q], f32)
        nc.vector.tensor_scalar(out=Mq[:], in0=io_q[:], scalar1=mq_c[:], scalar2=None, op0=OP.is_equal)
        Me = sing.tile([M, E], f32)
        nc.vector.tensor_scalar(out=Me[:], in0=io_e[:], scalar1=me_c[:], scalar2=None, op0=OP.is_equal)
        MeT = sing.tile([E, M], f32)
        nc.vector.tensor_scalar(out=MeT[:], in0=me_rr[:], scalar1=io_p96[:], scalar2=None, op0=OP.is_equal)
        MqTa = sing.tile([P, M], f32)
        nc.vector.tensor_scalar(out=MqTa[:], in0=mq_rra[:], scalar1=io_pa[:], scalar2=None, op0=OP.is_equal)
        MqTb = sing.tile([Q - P, M], f32)
        nc.vector.tensor_scalar(out=MqTb[:], in0=mq_rrb[:], scalar1=io_pb[:], scalar2=None, op0=OP.is_equal)

        # ---------- tiny matmul gathers ----------
        pts_ps = ps.tile([M, 2], f32)
        nc.tensor.matmul(out=pts_ps[:], lhsT=MeT[:], rhs=inc_sb[:], start=True, stop=True)
        ptsr = sing.tile([M, 2], f32)
        nc.vector.tensor_copy(out=ptsr[:], in_=pts_ps[:])

        cen_ps = ps.tile([M, 2], f32)
        nc.tensor.matmul(out=cen_ps[:], lhsT=MqTa[:], rhs=posa[:], start=True, stop=False)
        nc.tensor.matmul(out=cen_ps[:], lhsT=MqTb[:], rhs=posb[:], start=False, stop=True)
        cenr = sing.tile([M, 2], f32)
        nc.vector.tensor_copy(out=cenr[:], in_=cen_ps[:])

        chr_ps = ps.tile([M, 4], f32)
        nc.tensor.matmul(out=chr_ps[:], lhsT=MqTa[:], rhs=chola[:], start=True, stop=False)
        nc.tensor.matmul(out=chr_ps[:], lhsT=MqTb[:], rhs=cholb[:], start=False, stop=True)
        cholr = sing.tile([M, 4], f32)
        nc.vector.tensor_copy(out=cholr[:], in_=chr_ps[:])

        H_ps = ps.tile([E, Q], f32)
        nc.tensor.matmul(out=H_ps[:], lhsT=Me[:], rhs=Mq[:], start=True, stop=True)
        Hs = sing.tile([E, Q], f32)
        nc.vector.tensor_copy(out=Hs[:], in_=H_ps[:])

        # matched-q indicator, replicated to all partitions (column sums of Mq)
        ind_ps = ps.tile([P, Q], f32)
        nc.tensor.matmul(out=ind_ps[:], lhsT=onesw[:], rhs=Mq[:], start=True, stop=True)
        ind_bf = sing.tile([P, Q], bf16)
        nc.vector.tensor_copy(out=ind_bf[:], in_=ind_ps[:])
        ind1 = sing.tile([1, Q], f32)
        nc.vector.tensor_copy(out=ind1[:], in_=ind_ps[0:1, :])

        # ---------- window offsets ----------
        rmag = sing.tile([M, 2], f32)
        nc.vector.tensor_scalar(out=rmag[:], in0=ptsr[:], scalar1=MAGIC, scalar2=-MAGIC,
                                op0=OP.add, op1=OP.add)
        gtm = sing.tile([M, 2], f32)
        nc.vector.tensor_tensor(out=gtm[:], in0=rmag[:], in1=ptsr[:], op=OP.is_gt)
        pixf = sing.tile([M, 2], f32)
        nc.vector.tensor_tensor(out=pixf[:], in0=rmag[:], in1=gtm[:], op=OP.subtract)
        base = sing.tile([M, 1], f32)
        nc.vector.tensor_scalar(out=base[:], in0=pixf[:, 0:1], scalar1=float(W),
                                scalar2=float(-3 * W - 3), op0=OP.mult, op1=OP.add)
        nc.vector.tensor_tensor(out=base[:], in0=base[:], in1=pixf[:, 1:2], op=OP.add)
        sofs = sing.tile([M, WIN], f32)
        nc.vector.tensor_scalar(out=sofs[:], in0=drof[:], scalar1=base[:], scalar2=rbn[:],
                                op0=OP.add, op1=OP.add)
        v1 = sing.tile([M, WIN], f32)
        nc.vector.tensor_scalar(out=v1[:], in0=sofs[:], scalar1=0.0, scalar2=None, op0=OP.is_ge)
        v2 = sing.tile([M, WIN], f32)
        nc.vector.tensor_scalar(out=v2[:], in0=sofs[:], scalar1=float(NPIX - 1), scalar2=None, op0=OP.is_le)
        valid = sing.tile([M, WIN], f32)
        nc.vector.tensor_tensor(out=valid[:], in0=v1[:], in1=v2[:], op=OP.mult)
        clam = sing.tile([M, WIN], f32)
        nc.vector.tensor_scalar(out=clam[:], in0=sofs[:], scalar1=0.0, scalar2=float(NPIX - WIN),
                                op0=OP.max, op1=OP.min)
        # element offsets: clamped_flat_pixel * C + matched channel (< 2^23, exact)
        soft = sing.tile([M, WIN], f32)
        nc.vector.tensor_scalar(out=soft[:], in0=clam[:], scalar1=float(E), scalar2=me_c[:],
                                op0=OP.mult, op1=OP.add)
        nc.vector.tensor_scalar(out=soft[:], in0=soft[:], scalar1=MAGIC, scalar2=None, op0=OP.add)
        soft_i = sing.tile([M, WIN], i32)
        nc.vector.tensor_scalar(out=soft_i[:], in0=soft[:].bitcast(i32), scalar1=0x007FFFFF,
                                scalar2=None, op0=OP.bitwise_and)
        sofb = sing.tile([M, WIN], f32)
        nc.vector.tensor_scalar(out=sofb[:], in0=clam[:], scalar1=float(Q), scalar2=mq_c[:],
                                op0=OP.mult, op1=OP.add)
        nc.vector.tensor_scalar(out=sofb[:], in0=sofb[:], scalar1=MAGIC, scalar2=None, op0=OP.add)
        sofb_i = sing.tile([M, WIN], i32)
        nc.vector.tensor_scalar(out=sofb_i[:], in0=sofb[:].bitcast(i32), scalar1=0x007FFFFF,
                                scalar2=None, op0=OP.bitwise_and)

        # ---------- window gathers (one indirect DMA per window row) ----------
        WINDOWS_ON = "win" not in DIS
        RUNT = (WIN - 1) * E + 1
        RUNB = (WIN - 1) * Q + 1
        tw = sing.tile([M, WIN, RUNT], f32)
        bw = sing.tile([M, WIN, RUNB], f32)
        true_flat = bass.AP(tensor=true_sl.ap().tensor, offset=0, ap=[[1, NPIX * E], [1, 1]])
        bin_flat = bass.AP(tensor=bin_sl.ap().tensor, offset=0, ap=[[1, NPIX * Q], [1, 1]])
        for dr in range(WIN if WINDOWS_ON else 0):
            nc.gpsimd.indirect_dma_start(
                out=tw[:, dr, :], out_offset=None, in_=true_flat,
                in_offset=bass.IndirectOffsetOnAxis(ap=soft_i[:, dr:dr + 1], axis=0))
            nc.gpsimd.indirect_dma_start(
                out=bw[:, dr, :], out_offset=None, in_=bin_flat,
                in_offset=bass.IndirectOffsetOnAxis(ap=sofb_i[:, dr:dr + 1], axis=0))

        if not WINDOWS_ON:
            nc.vector.memset(tw[:], 0.0)
            nc.vector.memset(bw[:], 0.0)
        # ---------- dice streaming ----------
        por_v = por_sl.ap().rearrange("(p j) q -> p j q", p=P)
        true_v = true_sl.ap().rearrange("(p j) e -> p j e", p=P)
        C_ps = ps.tile([E, Q], f32)
        for c in range(NCHUNK):
            sl = slice(c * JC, (c + 1) * JC)
            por_t = big.tile([P, JC, Q], f32, tag="por")
            nc.sync.dma_start(out=por_t[:], in_=por_v[:, sl, :])
            true_t = big.tile([P, JC, E], f32, tag="true")
            nc.sync.dma_start(out=true_t[:], in_=true_v[:, sl, :])
            exp_t = big.tile([P, JC, Q], bf16, tag="exp")
            nc.scalar.activation(out=exp_t[:], in_=por_t[:], func=AF.Exp)
            nc.vector.tensor_tensor(out=exp_t[:], in0=exp_t[:], in1=bc(ind_bf[:], 1, JC), op=OP.mult)
            z_t = big.tile([P, JC], f32, tag="z")
            z_eng = nc.gpsimd if (os.environ.get("GPZ") and c % 2 == 1) else nc.vector
            z_eng.reduce_sum(out=z_t[:], in_=exp_t[:], axis=AX.X)
            rz_t = big.tile([P, JC], f32, tag="rz")
            nc.vector.reciprocal(out=rz_t[:], in_=z_t[:])
            a_t = big.tile([P, JC, E], bf16, tag="a")
            a_inst = nc.vector.tensor_tensor(out=a_t[:], in0=true_t[:], in1=bc(rz_t[:], 2, E), op=OP.mult)
            if c == NCHUNK - 1:
                last_dice_dve = a_inst
            for kb in range(JC if "mm" not in DIS else 0):
                nc.tensor.matmul(out=C_ps[:], lhsT=a_t[:, kb, :], rhs=exp_t[:, kb, :],
                                 start=(c == 0 and kb == 0),
                                 stop=(c == NCHUNK - 1 and kb == JC - 1))
        if "mm" in DIS:
            nc.tensor.matmul(out=C_ps[:], lhsT=a_t[:, 0, :], rhs=exp_t[:, 0, :],
                             start=True, stop=True)

        Cs = sing.tile([E, Q], f32)
        nc.vector.tensor_copy(out=Cs[:], in_=C_ps[:])
        # C's rhs was masked exp, so sum_q C[e,q] = sum_pixels true[p,e] (the
        # 1/Z in the stationary cancels the masked-exp row sums): den for free.
        nc.vector.reduce_sum(out=stats[0:E, 3:4], in_=Cs[:], axis=AX.X)
        scr_c = sing.tile([E, Q], f32)
        nc.vector.tensor_tensor(out=scr_c[:], in0=Cs[:], in1=Hs[:], op=OP.mult)
        nc.vector.reduce_sum(out=stats[0:M, 2:3], in_=scr_c[:], axis=AX.X)

        # ---------- occupancy CE ----------
        occ_v = occ_sl.ap().rearrange("(p j) k -> p j k", p=P)
        occ_t = sing.tile([P, J, K], f32)
        nc.sync.dma_start(out=occ_t[:], in_=occ_v)
        oct_t = sing.tile([P, J], f32)
        nc.sync.dma_start(out=oct_t[:], in_=occt_f.ap())
        e4 = sing.tile([P, J, K], f32)
        nc.scalar.activation(out=e4[:], in_=occ_t[:], func=AF.Exp)
        s4 = sing.tile([P, J], f32)
        nc.vector.reduce_sum(out=s4[:], in_=e4[:], axis=AX.X)
        lse = sing.tile([P, J], f32)
        emit_ln("occ", lse[:], s4[:], P, J)
        xt = sing.tile([P, J], f32)
        mk = sing.tile([P, J], f32)
        pk = sing.tile([P, J], f32)
        for k in range(K):
            nc.vector.tensor_scalar(out=mk[:], in0=oct_t[:], scalar1=float(k), scalar2=None, op0=OP.is_equal)
            if k == 0:
                nc.vector.tensor_tensor(out=xt[:], in0=mk[:], in1=occ_t[:, :, k], op=OP.mult)
            else:
                nc.vector.tensor_tensor(out=pk[:], in0=mk[:], in1=occ_t[:, :, k], op=OP.mult)
                nc.vector.tensor_tensor(out=xt[:], in0=xt[:], in1=pk[:], op=OP.add)
        nc.vector.tensor_tensor(out=lse[:], in0=lse[:], in1=xt[:], op=OP.subtract)
        nc.vector.reduce_sum(out=stats[:, 4:5], in_=lse[:], axis=AX.X)

        # ---------- class loss (partition 0) ----------
        sp = sing.tile([1, Q], f32)
        emit_softplus("cls", sp[:], iel[:], 1, Q)
        t9 = sing.tile([1, Q], f32)
        nc.vector.tensor_scalar(out=t9[:], in0=sp[:], scalar1=0.9, scalar2=None, op0=OP.mult)
        nc.vector.tensor_tensor(out=t9[:], in0=t9[:], in1=iel[:], op=OP.subtract)
        scr_q = sing.tile([1, Q], f32)
        clsm = sing.tile([1, 1], f32)
        nc.vector.tensor_tensor(out=scr_q[:], in0=t9[:], in1=ind1[:], op=OP.mult)
        nc.vector.reduce_sum(out=clsm[:], in_=scr_q[:], axis=AX.X)
        spsum = sing.tile([1, 1], f32)
        nc.vector.reduce_sum(out=spsum[:], in_=sp[:], axis=AX.X)
        nc.vector.tensor_scalar(out=spsum[:], in0=spsum[:], scalar1=NO_E, scalar2=None, op0=OP.mult)
        nc.vector.tensor_tensor(out=res[:, 6:7], in0=spsum[:], in1=clsm[:], op=OP.add)

        # ---------- NLL (96 partitions) ----------
        d_ = sing.tile([M, 2], f32)
        nc.vector.tensor_tensor(out=d_[:], in0=ptsr[:], in1=cenr[:], op=OP.subtract)
        r00 = sing.tile([M, 1], f32)
        nc.vector.reciprocal(out=r00[:], in_=cholr[:, 0:1])
        r11 = sing.tile([M, 1], f32)
        nc.vector.reciprocal(out=r11[:], in_=cholr[:, 3:4])
        z0 = sing.tile([M, 1], f32)
        nc.vector.tensor_tensor(out=z0[:], in0=d_[:, 0:1], in1=r00[:], op=OP.mult)
        t1 = sing.tile([M, 1], f32)
        nc.vector.tensor_tensor(out=t1[:], in0=cholr[:, 2:3], in1=z0[:], op=OP.mult)
        nc.vector.tensor_tensor(out=t1[:], in0=d_[:, 1:2], in1=t1[:], op=OP.subtract)
        z1 = sing.tile([M, 1], f32)
        nc.vector.tensor_tensor(out=z1[:], in0=t1[:], in1=r11[:], op=OP.mult)
        sq = sing.tile([M, 1], f32)
        nc.vector.tensor_tensor(out=sq[:], in0=z0[:], in1=z0[:], op=OP.mult)
        sq1 = sing.tile([M, 1], f32)
        nc.vector.tensor_tensor(out=sq1[:], in0=z1[:], in1=z1[:], op=OP.mult)
        nc.vector.tensor_tensor(out=sq[:], in0=sq[:], in1=sq1[:], op=OP.add)
        ldet = sing.tile([M, 1], f32)
        nc.vector.tensor_tensor(out=ldet[:], in0=cholr[:, 0:1], in1=cholr[:, 3:4], op=OP.mult)
        lnd = sing.tile([M, 1], f32)
        emit_ln("nld", lnd[:], ldet[:], M, 1)
        nc.vector.tensor_scalar(out=sq[:], in0=sq[:], scalar1=0.5,
                                scalar2=float(np.log(2.0 * np.pi)), op0=OP.mult, op1=OP.add)
        nc.vector.tensor_tensor(out=stats[0:M, 0:1], in0=sq[:], in1=lnd[:], op=OP.add)

        # ---------- window extraction + bce ----------
        def restride_last(ap, step, count):
            new_ap = list(ap.ap)
            new_ap[-1] = [step, count]
            return bass.AP(tensor=ap.tensor, offset=ap.offset, ap=new_ap)

        from concourse.tile import add_dep_helper
        tv = sing.tile([M, WIN * WIN], f32)
        tv_i = nc.vector.tensor_copy(out=tv[:].rearrange("m (a b) -> m a b", a=WIN),
                                     in_=restride_last(tw[:], E, WIN))
        lg = sing.tile([M, WIN * WIN], f32)
        lg_i = nc.vector.tensor_copy(out=lg[:].rearrange("m (a b) -> m a b", a=WIN),
                                     in_=restride_last(bw[:], Q, WIN))
        # keep the gather-dependent extraction out of the dice DVE stream: it
        # must not head-of-line block DVE behind the indirect-DMA drain
        add_dep_helper(tv_i.ins, last_dice_dve.ins, reason="extract after dice")
        add_dep_helper(lg_i.ins, last_dice_dve.ins, reason="extract after dice")
        spw = sing.tile([M, WIN * WIN], f32)
        emit_softplus("win", spw[:], lg[:], M, WIN * WIN)
        prw = sing.tile([M, WIN * WIN], f32)
        nc.vector.tensor_tensor(out=prw[:], in0=lg[:], in1=tv[:], op=OP.mult)
        nc.vector.tensor_tensor(out=spw[:], in0=spw[:], in1=prw[:], op=OP.subtract)
        scr_w = sing.tile([M, WIN * WIN], f32)
        valid49 = sing.tile([M, WIN * WIN], f32)
        nc.vector.tensor_copy(out=valid49[:].rearrange("m (a b) -> m a b", a=WIN),
                              in_=bc(valid[:], 2, WIN))
        nc.vector.tensor_tensor(out=scr_w[:], in0=spw[:], in1=valid49[:], op=OP.mult)
        nc.vector.reduce_sum(out=stats[0:M, 1:2], in_=scr_w[:], axis=AX.X)

        # ---------- final cross-partition reduction ----------
        fin_ps = ps.tile([1, 6], f32)
        nc.tensor.matmul(out=fin_ps[:], lhsT=ones[:], rhs=stats[:], start=True, stop=True)
        nc.vector.tensor_copy(out=res[:, 0:6], in_=fin_ps[:])
        nc.sync.dma_start(out=partials.ap(), in_=res[:])

    nc.compile()
    return nc


def _get_nc():
    if "nc" not in _CACHE:
        _CACHE["nc"] = _build_nc()
    return _CACHE["nc"]


def make_in_maps(is_electron_logit, true_segmap, binary_mask_logits, portion_logits,
                 incidence_points, positions, chol, occupancy_logits, occupancy_true,
                 matched_q, matched_e):
    f = np.float32
    iota_q = np.tile(np.arange(Q, dtype=f), (M, 1))
    iota_e = np.tile(np.arange(E, dtype=f), (M, 1))
    io_pa = np.arange(P, dtype=f).reshape(P, 1)
    io_pb = np.arange(P, Q, dtype=f).reshape(Q - P, 1)
    io_p96 = np.arange(E, dtype=f).reshape(E, 1)
    drof = np.tile((np.arange(WIN, dtype=f) * W), (M, 1))
    in_maps = []
    for c in range(8):
        b, h = c // 2, c % 2
        sl = slice(h * HALF, (h + 1) * HALF)
        me = np.asarray(matched_e[b])
        mq = np.asarray(matched_q[b])
        chol_b = np.asarray(chol[b], dtype=f).reshape(Q, 4)
        pos_b = np.asarray(positions[b], dtype=f)
        in_maps.append(dict(
            true_sl=np.ascontiguousarray(true_segmap[b, sl]).reshape(NPIX, E),
            por_sl=np.ascontiguousarray(portion_logits[b, sl]).reshape(NPIX, Q),
            bin_sl=np.ascontiguousarray(binary_mask_logits[b, sl]).reshape(NPIX, Q),
            occ_sl=np.ascontiguousarray(occupancy_logits[b, sl]).reshape(NPIX, K),
            occt_f=np.ascontiguousarray(occupancy_true[b, sl]).reshape(P, J).astype(f),
            me_colf=me.astype(f).reshape(M, 1),
            mq_colf=mq.astype(f).reshape(M, 1),
            me_row_rep=np.tile(me.astype(f), (E, 1)),
            mq_row_rep_a=np.tile(mq.astype(f), (P, 1)),
            mq_row_rep_b=np.tile(mq.astype(f), (Q - P, 1)),
            iota_q_row=iota_q, iota_e_row=iota_e,
            iota_p_a=io_pa, iota_p_b=io_pb, iota_p_96=io_p96,
            drof_rep=drof,
            inc_pts=np.asarray(incidence_points[b], dtype=f),
            pos_a=pos_b[:P], pos_b=pos_b[P:],
            chol_a=chol_b[:P], chol_b=chol_b[P:],
            iel_row=np.asarray(is_electron_logit, dtype=f).reshape(B, Q)[b].reshape(1, Q),
            rb_neg=np.full((M, 1), -h * NPIX, dtype=f),
        ))
    return in_maps


def combine(partials_list):
    s = np.stack([np.asarray(p, dtype=np.float64).reshape(8) for p in partials_list])
    # slots: 0=nll_sum 1=bce_sum 2=num2_sum 3=den_true_sum 4=occ_sum 6=class_sum
    class_loss = s[0::2, 6].sum() / (B * Q)
    nll_loss = s[0::2, 0].sum() / (B * M)
    bce_loss = s[:, 1].sum() / (B * M * WIN * WIN)
    occ_loss = s[:, 4].sum() / (B * H * W)
    dice = 0.0
    for b in range(B):
        num = 2.0 * (s[2 * b, 2] + s[2 * b + 1, 2])
        den = s[2 * b, 3] + s[2 * b + 1, 3] + H * W
        dice += 1.0 - (num + 1.0) / (den + 1.0)
    dice_loss = dice / B
    return np.float32(class_loss + bce_loss + dice_loss + nll_loss + occ_loss)


def kernel(**inputs):
    from concourse.bass_utils import run_bass_kernel_spmd
    nc = _get_nc()
    in_maps = make_in_maps(**{k: np.asarray(v) for k, v in inputs.items()})
    r = run_bass_kernel_spmd(nc, in_maps, list(range(8)))
    return combine([r.results[c]["partials"] for c in range(8)])



# revision 2
# speedup vs baseline: 1.0437x; 1.0437x over previous
"""Trainium2 Bass kernel for nn_Criterion_32830730011569.

Strategy: 8 cores = (image b in 0..3) x (H-half h in 0..1). The host
pre-gathers the matched channels (true_r = true[..., me], por_r = por[..., mq])
and ships the per-core pixel slices as fp8(e3m4), so each core streams only
[18432, 96] x 2 plus the tiny occupancy tensors (~3.9MB vs 19MB of f32).

Dice per chunk of 24 pixel-rows: ACT computes exp(por_r) writing a transposed
[P, m, j] tile so the per-pixel 1/Z product runs in the DVE 2x bf16 mode with
the [P, j] reciprocal broadcast across the middle (m) axis. PE accumulates
C[m_t, m_e] = sum_pix true_r * softmax over one PSUM bank; trace(C) is the
dice numerator and the full sum of C is sum(true_r) (softmax rows sum to 1),
giving the denominator for free.

The 7x7-window BCE gathers one contiguous 1159-pixel run per matched electron
from host-built channel-major [M, H*W] images (222KB total vs 4.1MB), then
extracts the 7x7 with a strided copy. Occupancy CE streams fp8 logits plus the
host-pre-gathered label logit. NLL/class stay f32 on pre-gathered small
tensors. ln/exp both come from the natural_log_exp ACT table set (one load).
Each core returns 7 partial sums; the host combines them into the scalar loss.
"""
import sys

sys.path.insert(0, "/opt/trn_rl_repo")
import numpy as np

B, H, W, Q, E, M, K, WIN = 4, 192, 192, 160, 96, 96, 4, 7
NO_E = 0.1
HALF = H // 2          # rows per core slice
NPIX = HALF * W        # 18432 pixels per slice
FULLPIX = H * W        # 36864 pixels per image
P = 128                # partitions
J = NPIX // P          # 144 pixels per partition (p-major)
JC = 24                # pixels per chunk per partition
NCHUNK = J // JC       # 6
RUN = (WIN - 1) * W + WIN  # 1159: contiguous window-row span
MAGIC = 8388608.0      # 2^23

_CACHE = {}


def _build_nc():
    import concourse.bass as bass
    import concourse.bacc as bacc
    import concourse.tile as tile
    from concourse import mybir

    f32 = mybir.dt.float32
    i32 = mybir.dt.int32
    bf16 = mybir.dt.bfloat16
    f8 = mybir.dt.float8e3
    AF = mybir.ActivationFunctionType
    OP = mybir.AluOpType
    AX = mybir.AxisListType

    nc = bacc.Bacc("TRN2", target_bir_lowering=False, debug=False, num_devices=8)

    # ---- external I/O ----
    por_sl = nc.dram_tensor("por_sl", [NPIX, M], f8, kind="ExternalInput")
    true_sl = nc.dram_tensor("true_sl", [NPIX, M], f8, kind="ExternalInput")
    occ_sl = nc.dram_tensor("occ_sl", [P, J, K], f8, kind="ExternalInput")
    xsel_d = nc.dram_tensor("xsel", [P, J], bf16, kind="ExternalInput")
    twin = nc.dram_tensor("twin", [M, FULLPIX], f8, kind="ExternalInput")
    bwin = nc.dram_tensor("bwin", [M, FULLPIX], f8, kind="ExternalInput")
    ptsr_d = nc.dram_tensor("ptsr", [M, 2], f32, kind="ExternalInput")
    cenr_d = nc.dram_tensor("cenr", [M, 2], f32, kind="ExternalInput")
    cholr_d = nc.dram_tensor("cholr", [M, 4], f32, kind="ExternalInput")
    iel_d = nc.dram_tensor("iel", [1, Q], f32, kind="ExternalInput")
    lab_d = nc.dram_tensor("lab", [1, Q], f32, kind="ExternalInput")
    dr7_d = nc.dram_tensor("dr7", [M, WIN], f32, kind="ExternalInput")
    wlo_d = nc.dram_tensor("wlo", [M, 1], f32, kind="ExternalInput")
    whi_d = nc.dram_tensor("whi", [M, 1], f32, kind="ExternalInput")
    mbase_d = nc.dram_tensor("mbase", [M, 1], f32, kind="ExternalInput")
    ident_d = nc.dram_tensor("ident", [M, M], f32, kind="ExternalInput")
    partials = nc.dram_tensor("partials", [1, 8], f32, kind="ExternalOutput")

    def bc(ap, pos, count):
        """Insert a stride-0 broadcast dim into an AP at free-dim position pos."""
        new = list(ap.ap)
        new.insert(pos, [0, count])
        return bass.AP(tensor=ap.tensor, offset=ap.offset, ap=new)

    def restride(ap, dims):
        """Replace the free dims of a 2D AP with explicit [step, count] pairs."""
        new_ap = [ap.ap[0]] + [list(d) for d in dims]
        return bass.AP(tensor=ap.tensor, offset=ap.offset, ap=new_ap)

    from contextlib import ExitStack

    with tile.TileContext(nc) as tc, ExitStack() as ctx:
        sing = ctx.enter_context(tc.tile_pool(name="sing", bufs=1))
        big = ctx.enter_context(tc.tile_pool(name="big", bufs=2))
        ps = ctx.enter_context(tc.tile_pool(name="ps", bufs=1, space="PSUM"))

        def load(dram, shape, dt=f32):
            nm = dram.name + "_sb"
            t = sing.tile(shape, dt, name=nm, tag=nm)
            nc.sync.dma_start(out=t[:], in_=dram.ap())
            return t

        ptsr = load(ptsr_d, [M, 2])
        cenr = load(cenr_d, [M, 2])
        cholr = load(cholr_d, [M, 4])
        iel = load(iel_d, [1, Q])
        lab = load(lab_d, [1, Q])
        dr7 = load(dr7_d, [M, WIN])
        wlo = load(wlo_d, [M, 1])
        whi = load(whi_d, [M, 1])
        mbase = load(mbase_d, [M, 1])
        ident = load(ident_d, [M, M])

        ones = sing.tile([P, 1], f32)
        nc.vector.memset(ones[:], 1.0)
        stats = sing.tile([P, 6], f32)
        nc.vector.memset(stats[:], 0.0)
        res = sing.tile([1, 8], f32)
        nc.vector.memset(res[:], 0.0)

        # ---------- window offsets (floor via 2^23 magic) ----------
        rmag = sing.tile([M, 2], f32)
        nc.vector.tensor_scalar(out=rmag[:], in0=ptsr[:], scalar1=MAGIC, scalar2=-MAGIC,
                                op0=OP.add, op1=OP.add)
        gtm = sing.tile([M, 2], f32)
        nc.vector.tensor_tensor(out=gtm[:], in0=rmag[:], in1=ptsr[:], op=OP.is_gt)
        pixf = sing.tile([M, 2], f32)
        nc.vector.tensor_tensor(out=pixf[:], in0=rmag[:], in1=gtm[:], op=OP.subtract)
        # run start in full-image flat pixels: (r-3)*W + (c-3); always in
        # bounds because incidence points live in [4, 187]
        base = sing.tile([M, 1], f32)
        nc.vector.tensor_scalar(out=base[:], in0=pixf[:, 0:1], scalar1=float(W),
                                scalar2=float(-3 * W - 3), op0=OP.mult, op1=OP.add)
        nc.vector.tensor_tensor(out=base[:], in0=base[:], in1=pixf[:, 1:2], op=OP.add)
        soff = sing.tile([M, 1], f32)
        nc.vector.tensor_scalar(out=soff[:], in0=base[:], scalar1=mbase[:], scalar2=MAGIC,
                                op0=OP.add, op1=OP.add)
        soff_i = sing.tile([M, 1], i32)
        nc.vector.tensor_scalar(out=soff_i[:], in0=soff[:].bitcast(i32), scalar1=0x007FFFFF,
                                scalar2=None, op0=OP.bitwise_and)
        # per-window-row validity: absolute row r-3+dr inside this core's half
        p0m3 = sing.tile([M, 1], f32)
        nc.vector.tensor_scalar(out=p0m3[:], in0=pixf[:, 0:1], scalar1=-3.0, scalar2=None,
                                op0=OP.add)
        rows7 = sing.tile([M, WIN], f32)
        nc.vector.tensor_scalar(out=rows7[:], in0=dr7[:], scalar1=p0m3[:], scalar2=None,
                                op0=OP.add)
        v1 = sing.tile([M, WIN], f32)
        nc.vector.tensor_scalar(out=v1[:], in0=rows7[:], scalar1=wlo[:], scalar2=None, op0=OP.is_ge)
        v2 = sing.tile([M, WIN], f32)
        nc.vector.tensor_scalar(out=v2[:], in0=rows7[:], scalar1=whi[:], scalar2=None, op0=OP.is_le)
        valid = sing.tile([M, WIN], f32)
        nc.vector.tensor_tensor(out=valid[:], in0=v1[:], in1=v2[:], op=OP.mult)

        # ---------- window gathers: one 1159-element run per matched electron ----------
        twr = sing.tile([M, RUN], f8)
        bwr = sing.tile([M, RUN], f8)
        twin_flat = bass.AP(tensor=twin.ap().tensor, offset=0, ap=[[1, M * FULLPIX], [1, 1]])
        bwin_flat = bass.AP(tensor=bwin.ap().tensor, offset=0, ap=[[1, M * FULLPIX], [1, 1]])
        nc.gpsimd.indirect_dma_start(
            out=twr[:], out_offset=None, in_=twin_flat,
            in_offset=bass.IndirectOffsetOnAxis(ap=soff_i[:, 0:1], axis=0))
        nc.gpsimd.indirect_dma_start(
            out=bwr[:], out_offset=None, in_=bwin_flat,
            in_offset=bass.IndirectOffsetOnAxis(ap=soff_i[:, 0:1], axis=0))

        # ---------- dice streaming ----------
        por_v = por_sl.ap().rearrange("(p j) m -> p j m", p=P)
        true_v = true_sl.ap().rearrange("(p j) m -> p j m", p=P)
        C_ps = ps.tile([M, M], f32)
        for c in range(NCHUNK):
            sl = slice(c * JC, (c + 1) * JC)
            por_t = big.tile([P, JC, M], f8, tag="por")
            nc.sync.dma_start(out=por_t[:], in_=por_v[:, sl, :])
            t_t = big.tile([P, JC, M], f8, tag="t")
            nc.sync.dma_start(out=t_t[:], in_=true_v[:, sl, :])
            expT = big.tile([P, M, JC], bf16, tag="expT")
            nc.scalar.activation(out=expT[:].rearrange("p m j -> p j m"), in_=por_t[:],
                                 func=AF.Exp)
            z_t = big.tile([P, JC], f32, tag="z")
            nc.vector.reduce_sum(out=z_t[:], in_=expT[:].rearrange("p m j -> p j m"),
                                 axis=AX.X)
            rz = big.tile([P, JC], f32, tag="rz")
            nc.vector.reciprocal(out=rz[:], in_=z_t[:])
            rzb = big.tile([P, JC], bf16, tag="rzb")
            nc.vector.tensor_copy(out=rzb[:], in_=rz[:])
            epT = big.tile([P, M, JC], bf16, tag="epT")
            nc.vector.tensor_tensor(out=epT[:], in0=expT[:], in1=bc(rzb[:], 1, M), op=OP.mult)
            for j in range(JC):
                nc.tensor.matmul(out=C_ps[:], lhsT=t_t[:, j, :], rhs=epT[:, :, j],
                                 start=(c == 0 and j == 0),
                                 stop=(c == NCHUNK - 1 and j == JC - 1))

        Cs = sing.tile([M, M], f32)
        nc.vector.tensor_copy(out=Cs[:], in_=C_ps[:])
        # trace(C) = sum_pix sum_m true_r * portion (the dice numerator / 2);
        # sum of C = sum_pix true_r since each softmax row of ep sums to 1.
        scr_c = sing.tile([M, M], f32)
        nc.vector.tensor_tensor(out=scr_c[:], in0=Cs[:], in1=ident[:], op=OP.mult)
        nc.vector.reduce_sum(out=stats[0:M, 2:3], in_=scr_c[:], axis=AX.X)
        nc.vector.reduce_sum(out=stats[0:M, 3:4], in_=Cs[:], axis=AX.X)

        # ---------- occupancy CE ----------
        occ_t = sing.tile([P, J, K], f8)
        nc.sync.dma_start(out=occ_t[:], in_=occ_sl.ap())
        xsel_t = sing.tile([P, J], bf16)
        nc.sync.dma_start(out=xsel_t[:], in_=xsel_d.ap())
        e4 = sing.tile([P, J, K], bf16)
        nc.scalar.activation(out=e4[:], in_=occ_t[:], func=AF.Exp)
        s4 = sing.tile([P, J], f32)
        nc.vector.reduce_sum(out=s4[:], in_=e4[:], axis=AX.X)
        lse = sing.tile([P, J], f32)
        nc.scalar.activation(out=lse[:], in_=s4[:], func=AF.Ln)
        d4 = sing.tile([P, J], f32)
        nc.vector.tensor_tensor(out=d4[:], in0=lse[:], in1=xsel_t[:], op=OP.subtract)
        nc.vector.reduce_sum(out=stats[:, 4:5], in_=d4[:], axis=AX.X)

        # ---------- class loss (partition 0) ----------
        expc = sing.tile([1, Q], f32)
        nc.scalar.activation(out=expc[:], in_=iel[:], func=AF.Exp)
        nc.vector.tensor_scalar(out=expc[:], in0=expc[:], scalar1=1.0, scalar2=None, op0=OP.add)
        sp = sing.tile([1, Q], f32)
        nc.scalar.activation(out=sp[:], in_=expc[:], func=AF.Ln)
        t9 = sing.tile([1, Q], f32)
        nc.vector.tensor_scalar(out=t9[:], in0=sp[:], scalar1=0.9, scalar2=None, op0=OP.mult)
        nc.vector.tensor_tensor(out=t9[:], in0=t9[:], in1=iel[:], op=OP.subtract)
        scr_q = sing.tile([1, Q], f32)
        nc.vector.tensor_tensor(out=scr_q[:], in0=t9[:], in1=lab[:], op=OP.mult)
        clsm = sing.tile([1, 1], f32)
        nc.vector.reduce_sum(out=clsm[:], in_=scr_q[:], axis=AX.X)
        spsum = sing.tile([1, 1], f32)
        nc.vector.reduce_sum(out=spsum[:], in_=sp[:], axis=AX.X)
        nc.vector.tensor_scalar(out=spsum[:], in0=spsum[:], scalar1=NO_E, scalar2=None, op0=OP.mult)
        nc.vector.tensor_tensor(out=res[:, 6:7], in0=spsum[:], in1=clsm[:], op=OP.add)

        # ---------- NLL (96 partitions, f32 — the only term that needs precision) ----------
        d_ = sing.tile([M, 2], f32)
        nc.vector.tensor_tensor(out=d_[:], in0=ptsr[:], in1=cenr[:], op=OP.subtract)
        r00 = sing.tile([M, 1], f32)
        nc.vector.reciprocal(out=r00[:], in_=cholr[:, 0:1])
        r11 = sing.tile([M, 1], f32)
        nc.vector.reciprocal(out=r11[:], in_=cholr[:, 3:4])
        z0 = sing.tile([M, 1], f32)
        nc.vector.tensor_tensor(out=z0[:], in0=d_[:, 0:1], in1=r00[:], op=OP.mult)
        t1 = sing.tile([M, 1], f32)
        nc.vector.tensor_tensor(out=t1[:], in0=cholr[:, 2:3], in1=z0[:], op=OP.mult)
        nc.vector.tensor_tensor(out=t1[:], in0=d_[:, 1:2], in1=t1[:], op=OP.subtract)
        z1 = sing.tile([M, 1], f32)
        nc.vector.tensor_tensor(out=z1[:], in0=t1[:], in1=r11[:], op=OP.mult)
        sq = sing.tile([M, 1], f32)
        nc.vector.tensor_tensor(out=sq[:], in0=z0[:], in1=z0[:], op=OP.mult)
        sq1 = sing.tile([M, 1], f32)
        nc.vector.tensor_tensor(out=sq1[:], in0=z1[:], in1=z1[:], op=OP.mult)
        nc.vector.tensor_tensor(out=sq[:], in0=sq[:], in1=sq1[:], op=OP.add)
        ldet = sing.tile([M, 1], f32)
        nc.vector.tensor_tensor(out=ldet[:], in0=cholr[:, 0:1], in1=cholr[:, 3:4], op=OP.mult)
        lnd = sing.tile([M, 1], f32)
        nc.scalar.activation(out=lnd[:], in_=ldet[:], func=AF.Ln)
        nc.vector.tensor_scalar(out=sq[:], in0=sq[:], scalar1=0.5,
                                scalar2=float(np.log(2.0 * np.pi)), op0=OP.mult, op1=OP.add)
        nc.vector.tensor_tensor(out=stats[0:M, 0:1], in0=sq[:], in1=lnd[:], op=OP.add)

        # ---------- window extraction + bce ----------
        tv = sing.tile([M, WIN * WIN], f32)
        nc.vector.tensor_copy(out=tv[:].rearrange("m (a b) -> m a b", a=WIN),
                              in_=restride(twr[:], [[W, WIN], [1, WIN]]))
        lg = sing.tile([M, WIN * WIN], f32)
        nc.vector.tensor_copy(out=lg[:].rearrange("m (a b) -> m a b", a=WIN),
                              in_=restride(bwr[:], [[W, WIN], [1, WIN]]))
        expw = sing.tile([M, WIN * WIN], f32)
        nc.scalar.activation(out=expw[:], in_=lg[:], func=AF.Exp)
        nc.vector.tensor_scalar(out=expw[:], in0=expw[:], scalar1=1.0, scalar2=None, op0=OP.add)
        spw = sing.tile([M, WIN * WIN], f32)
        nc.scalar.activation(out=spw[:], in_=expw[:], func=AF.Ln)
        prw = sing.tile([M, WIN * WIN], f32)
        nc.vector.tensor_tensor(out=prw[:], in0=lg[:], in1=tv[:], op=OP.mult)
        nc.vector.tensor_tensor(out=spw[:], in0=spw[:], in1=prw[:], op=OP.subtract)
        valid49 = sing.tile([M, WIN * WIN], f32)
        nc.vector.tensor_copy(out=valid49[:].rearrange("m (a b) -> m a b", a=WIN),
                              in_=bc(valid[:], 2, WIN))
        scr_w = sing.tile([M, WIN * WIN], f32)
        nc.vector.tensor_tensor(out=scr_w[:], in0=spw[:], in1=valid49[:], op=OP.mult)
        nc.vector.reduce_sum(out=stats[0:M, 1:2], in_=scr_w[:], axis=AX.X)

        # ---------- final cross-partition reduction ----------
        fin_ps = ps.tile([1, 6], f32)
        nc.tensor.matmul(out=fin_ps[:], lhsT=ones[:], rhs=stats[:], start=True, stop=True)
        nc.vector.tensor_copy(out=res[:, 0:6], in_=fin_ps[:])
        nc.sync.dma_start(out=partials.ap(), in_=res[:])

    nc.compile()
    return nc


def _get_nc():
    if "nc" not in _CACHE:
        _CACHE["nc"] = _build_nc()
    return _CACHE["nc"]


def make_in_maps(is_electron_logit, true_segmap, binary_mask_logits, portion_logits,
                 incidence_points, positions, chol, occupancy_logits, occupancy_true,
                 matched_q, matched_e):
    import ml_dtypes
    f = np.float32
    f8 = ml_dtypes.float8_e3m4
    bf = ml_dtypes.bfloat16
    dr7 = np.tile(np.arange(WIN, dtype=f), (M, 1))
    mbase = (np.arange(M, dtype=f) * FULLPIX).reshape(M, 1)
    ident = np.eye(M, dtype=f)
    in_maps = []
    for b in range(B):
        me = np.asarray(matched_e[b])
        mq = np.asarray(matched_q[b])
        true_r = np.asarray(true_segmap[b])[:, :, me]          # [H, W, M]
        por_r = np.asarray(portion_logits[b])[:, :, mq]        # [H, W, M]
        bin_r = np.asarray(binary_mask_logits[b])[:, :, mq]    # [H, W, M]
        twin_b = np.ascontiguousarray(true_r.reshape(FULLPIX, M).T).astype(f8)
        bwin_b = np.ascontiguousarray(bin_r.reshape(FULLPIX, M).T).astype(f8)
        ptsr = np.asarray(incidence_points[b], dtype=f)[me]
        cenr = np.asarray(positions[b], dtype=f)[mq]
        cholr = np.asarray(chol[b], dtype=f).reshape(Q, 4)[mq]
        iel = np.asarray(is_electron_logit, dtype=f).reshape(B, Q)[b].reshape(1, Q)
        lab = np.zeros((1, Q), dtype=f)
        lab[0, mq] = 1.0
        occ_b = np.asarray(occupancy_logits[b], dtype=f)
        occt_b = np.asarray(occupancy_true[b])
        xsel_b = np.take_along_axis(occ_b.reshape(FULLPIX, K),
                                    occt_b.reshape(FULLPIX, 1), axis=1)
        for h in range(2):
            sl = slice(h * HALF, (h + 1) * HALF)
            psl = slice(h * NPIX, (h + 1) * NPIX)
            in_maps.append(dict(
                por_sl=np.ascontiguousarray(por_r[sl]).reshape(NPIX, M).astype(f8),
                true_sl=np.ascontiguousarray(true_r[sl]).reshape(NPIX, M).astype(f8),
                occ_sl=np.ascontiguousarray(occ_b[sl]).reshape(P, J, K).astype(f8),
                xsel=np.ascontiguousarray(xsel_b[psl]).reshape(P, J).astype(bf),
                twin=twin_b, bwin=bwin_b,
                ptsr=ptsr, cenr=cenr, cholr=cholr, iel=iel, lab=lab,
                dr7=dr7,
                wlo=np.full((M, 1), float(h * HALF), dtype=f),
                whi=np.full((M, 1), float(h * HALF + HALF - 1), dtype=f),
                mbase=mbase, ident=ident,
            ))
    return in_maps


def combine(partials_list):
    s = np.stack([np.asarray(p, dtype=np.float64).reshape(8) for p in partials_list])
    # slots: 0=nll_sum 1=bce_sum 2=trace(C) 3=sum(C)=sum_true 4=occ_sum 6=class_sum
    class_loss = s[0::2, 6].sum() / (B * Q)
    nll_loss = s[0::2, 0].sum() / (B * M)
    bce_loss = s[:, 1].sum() / (B * M * WIN * WIN)
    occ_loss = s[:, 4].sum() / (B * H * W)
    dice = 0.0
    for b in range(B):
        num = 2.0 * (s[2 * b, 2] + s[2 * b + 1, 2])
        den = s[2 * b, 3] + s[2 * b + 1, 3] + H * W
        dice += 1.0 - (num + 1.0) / (den + 1.0)
    dice_loss = dice / B
    return np.float32(class_loss + bce_loss + dice_loss + nll_loss + occ_loss)


def kernel(**inputs):
    from concourse.bass_utils import run_bass_kernel_spmd
    nc = _get_nc()
    in_maps = make_in_maps(**{k: np.asarray(v) for k, v in inputs.items()})
    r = run_bass_kernel_spmd(nc, in_maps, list(range(8)))
    return combine([r.results[c]["partials"] for c in range(8)])


# revision 10
# speedup vs baseline: 1.9523x; 1.8705x over previous
"""Trainium2 Bass kernel for nn_Criterion_32830730011569.

Strategy: 8 cores = (image b in 0..3) x (H-half h in 0..1). The host
pre-gathers the matched channels (true_r = true[..., me], por_r = por[..., mq])
and ships the per-core pixel slices as fp8(e3m4), so each core streams only
[18432, 96] x 2 plus the tiny occupancy tensors (~3.9MB vs 19MB of f32).

Dice per chunk of 24 pixel-rows: ACT computes exp(por_r) writing a transposed
[P, m, j] tile so the per-pixel 1/Z product runs in the DVE 2x bf16 mode with
the [P, j] reciprocal broadcast across the middle (m) axis. PE accumulates
C[m_t, m_e] = sum_pix true_r * softmax over one PSUM bank; trace(C) is the
dice numerator and the full sum of C is sum(true_r) (softmax rows sum to 1),
giving the denominator for free.

The 7x7-window BCE gathers one contiguous 1159-pixel run per matched electron
from host-built channel-major [M, H*W] images (222KB total vs 4.1MB), then
extracts the 7x7 with a strided copy. Occupancy CE streams fp8 logits plus the
host-pre-gathered label logit. NLL/class stay f32 on pre-gathered small
tensors. ln/exp both come from the natural_log_exp ACT table set (one load).
Each core returns 7 partial sums; the host combines them into the scalar loss.
"""
import sys

sys.path.insert(0, "/opt/trn_rl_repo")
import numpy as np

B, H, W, Q, E, M, K, WIN = 4, 192, 192, 160, 96, 96, 4, 7
NO_E = 0.1
HALF = H // 2          # rows per core slice
NPIX = HALF * W        # 18432 pixels per slice
FULLPIX = H * W        # 36864 pixels per image
P = 128                # partitions
J = NPIX // P          # 144 pixels per partition (p-major)
JC = 36                # pixels per chunk per partition
NCHUNK = J // JC       # 4
RUN = (WIN - 1) * W + WIN  # 1159: contiguous window-row span
MAGIC = 8388608.0      # 2^23

_CACHE = {}


def _build_nc():
    import concourse.bass as bass
    import concourse.bacc as bacc
    import concourse.tile as tile
    from concourse import mybir

    f32 = mybir.dt.float32
    i32 = mybir.dt.int32
    bf16 = mybir.dt.bfloat16
    f8 = mybir.dt.float8e3
    f8e4 = mybir.dt.float8e4
    AF = mybir.ActivationFunctionType
    OP = mybir.AluOpType
    AX = mybir.AxisListType
    DROW = mybir.MatmulPerfMode.DoubleRow

    nc = bacc.Bacc("TRN2", target_bir_lowering=False, debug=False, num_devices=8)

    # ---- external I/O ----
    por_sl = nc.dram_tensor("por_sl", [NPIX, M], f8, kind="ExternalInput")
    true_sl = nc.dram_tensor("true_sl", [NPIX, M], f8e4, kind="ExternalInput")
    occ_sl = nc.dram_tensor("occ_sl", [P, J, K], f8, kind="ExternalInput")
    xsel_d = nc.dram_tensor("xsel", [P, J], bf16, kind="ExternalInput")
    twin = nc.dram_tensor("twin", [M, FULLPIX], f8, kind="ExternalInput")
    bwin = nc.dram_tensor("bwin", [M, FULLPIX], f8, kind="ExternalInput")
    ptsr_d = nc.dram_tensor("ptsr", [M, 2], f32, kind="ExternalInput")
    cenr_d = nc.dram_tensor("cenr", [M, 2], f32, kind="ExternalInput")
    cholr_d = nc.dram_tensor("cholr", [M, 4], f32, kind="ExternalInput")
    iel_d = nc.dram_tensor("iel", [1, Q], f32, kind="ExternalInput")
    lab_d = nc.dram_tensor("lab", [1, Q], f32, kind="ExternalInput")
    dr7_d = nc.dram_tensor("dr7", [M, WIN], f32, kind="ExternalInput")
    wlo_d = nc.dram_tensor("wlo", [M, 1], f32, kind="ExternalInput")
    whi_d = nc.dram_tensor("whi", [M, 1], f32, kind="ExternalInput")
    mbase_d = nc.dram_tensor("mbase", [M, 1], f32, kind="ExternalInput")
    ident_d = nc.dram_tensor("ident", [M, M], f32, kind="ExternalInput")
    partials = nc.dram_tensor("partials", [1, 8], f32, kind="ExternalOutput")

    def bc(ap, pos, count):
        """Insert a stride-0 broadcast dim into an AP at free-dim position pos."""
        new = list(ap.ap)
        new.insert(pos, [0, count])
        return bass.AP(tensor=ap.tensor, offset=ap.offset, ap=new)

    def restride(ap, dims):
        """Replace the free dims of a 2D AP with explicit [step, count] pairs."""
        new_ap = [ap.ap[0]] + [list(d) for d in dims]
        return bass.AP(tensor=ap.tensor, offset=ap.offset, ap=new_ap)

    from contextlib import ExitStack

    with tile.TileContext(nc) as tc, ExitStack() as ctx:
        sing = ctx.enter_context(tc.tile_pool(name="sing", bufs=1))
        big = ctx.enter_context(tc.tile_pool(name="big", bufs=2))
        ps = ctx.enter_context(tc.tile_pool(name="ps", bufs=1, space="PSUM"))

        def load(dram, shape, dt=f32):
            nm = dram.name + "_sb"
            t = sing.tile(shape, dt, name=nm, tag=nm)
            nc.sync.dma_start(out=t[:], in_=dram.ap())
            return t

        ptsr = load(ptsr_d, [M, 2])
        cenr = load(cenr_d, [M, 2])
        cholr = load(cholr_d, [M, 4])
        iel = load(iel_d, [1, Q])
        lab = load(lab_d, [1, Q])
        dr7 = load(dr7_d, [M, WIN])
        wlo = load(wlo_d, [M, 1])
        whi = load(whi_d, [M, 1])
        mbase = load(mbase_d, [M, 1])
        ident = load(ident_d, [M, M])

        ones = sing.tile([P, 1], f32)
        nc.vector.memset(ones[:], 1.0)
        stats = sing.tile([P, 6], f32)
        nc.vector.memset(stats[:], 0.0)
        res = sing.tile([1, 8], f32)
        nc.vector.memset(res[:], 0.0)

        # ---------- window offsets (floor via 2^23 magic) ----------
        rmag = sing.tile([M, 2], f32)
        nc.vector.tensor_scalar(out=rmag[:], in0=ptsr[:], scalar1=MAGIC, scalar2=-MAGIC,
                                op0=OP.add, op1=OP.add)
        gtm = sing.tile([M, 2], f32)
        nc.vector.tensor_tensor(out=gtm[:], in0=rmag[:], in1=ptsr[:], op=OP.is_gt)
        pixf = sing.tile([M, 2], f32)
        nc.vector.tensor_tensor(out=pixf[:], in0=rmag[:], in1=gtm[:], op=OP.subtract)
        # run start in full-image flat pixels: (r-3)*W + (c-3); always in
        # bounds because incidence points live in [4, 187]
        base = sing.tile([M, 1], f32)
        nc.vector.tensor_scalar(out=base[:], in0=pixf[:, 0:1], scalar1=float(W),
                                scalar2=float(-3 * W - 3), op0=OP.mult, op1=OP.add)
        nc.vector.tensor_tensor(out=base[:], in0=base[:], in1=pixf[:, 1:2], op=OP.add)
        soff = sing.tile([M, 1], f32)
        nc.vector.tensor_scalar(out=soff[:], in0=base[:], scalar1=mbase[:], scalar2=MAGIC,
                                op0=OP.add, op1=OP.add)
        soff_i = sing.tile([M, 1], i32)
        nc.vector.tensor_scalar(out=soff_i[:], in0=soff[:].bitcast(i32), scalar1=0x007FFFFF,
                                scalar2=None, op0=OP.bitwise_and)
        # per-window-row validity: absolute row r-3+dr inside this core's half
        p0m3 = sing.tile([M, 1], f32)
        nc.vector.tensor_scalar(out=p0m3[:], in0=pixf[:, 0:1], scalar1=-3.0, scalar2=None,
                                op0=OP.add)
        rows7 = sing.tile([M, WIN], f32)
        nc.vector.tensor_scalar(out=rows7[:], in0=dr7[:], scalar1=p0m3[:], scalar2=None,
                                op0=OP.add)
        v1 = sing.tile([M, WIN], f32)
        nc.vector.tensor_scalar(out=v1[:], in0=rows7[:], scalar1=wlo[:], scalar2=None, op0=OP.is_ge)
        v2 = sing.tile([M, WIN], f32)
        nc.vector.tensor_scalar(out=v2[:], in0=rows7[:], scalar1=whi[:], scalar2=None, op0=OP.is_le)
        valid = sing.tile([M, WIN], f32)
        nc.vector.tensor_tensor(out=valid[:], in0=v1[:], in1=v2[:], op=OP.mult)

        # ---------- window gathers: one 1159-element run per matched electron ----------
        twr = sing.tile([M, RUN], f8)
        bwr = sing.tile([M, RUN], f8)
        twin_flat = bass.AP(tensor=twin.ap().tensor, offset=0, ap=[[1, M * FULLPIX], [1, 1]])
        bwin_flat = bass.AP(tensor=bwin.ap().tensor, offset=0, ap=[[1, M * FULLPIX], [1, 1]])
        nc.gpsimd.indirect_dma_start(
            out=twr[:], out_offset=None, in_=twin_flat,
            in_offset=bass.IndirectOffsetOnAxis(ap=soff_i[:, 0:1], axis=0))
        nc.gpsimd.indirect_dma_start(
            out=bwr[:], out_offset=None, in_=bwin_flat,
            in_offset=bass.IndirectOffsetOnAxis(ap=soff_i[:, 0:1], axis=0))

        # ---------- dice streaming ----------
        por_v = por_sl.ap().rearrange("(p j) m -> p j m", p=P)
        true_v = true_sl.ap().rearrange("(p j) m -> p j m", p=P)
        C_ps = ps.tile([M, M], f32)
        for c in range(NCHUNK):
            sl = slice(c * JC, (c + 1) * JC)
            por_t = big.tile([P, JC, M], f8, tag="por")
            nc.sync.dma_start(out=por_t[:], in_=por_v[:, sl, :])
            t_t = big.tile([P, JC, M], f8e4, tag="t")
            nc.sync.dma_start(out=t_t[:], in_=true_v[:, sl, :])
            exp_t = big.tile([P, JC, M], bf16, tag="exp")
            nc.scalar.activation(out=exp_t[:], in_=por_t[:], func=AF.Exp)
            # Z via pairwise-tree adds: every level runs in the DVE 2x bf16
            # mode, vs tensor_reduce which is always 1x
            zt1 = big.tile([P, JC, 48], bf16, tag="zt1")
            nc.vector.tensor_tensor(out=zt1[:], in0=exp_t[:, :, 0:48],
                                    in1=exp_t[:, :, 48:96], op=OP.add)
            zt2 = big.tile([P, JC, 24], bf16, tag="zt2")
            nc.vector.tensor_tensor(out=zt2[:], in0=zt1[:, :, 0:24],
                                    in1=zt1[:, :, 24:48], op=OP.add)
            zt3 = big.tile([P, JC, 12], bf16, tag="zt3")
            nc.vector.tensor_tensor(out=zt3[:], in0=zt2[:, :, 0:12],
                                    in1=zt2[:, :, 12:24], op=OP.add)
            zt4 = big.tile([P, JC, 6], bf16, tag="zt4")
            nc.vector.tensor_tensor(out=zt4[:], in0=zt3[:, :, 0:6],
                                    in1=zt3[:, :, 6:12], op=OP.add)
            zt5 = big.tile([P, JC, 3], f32, tag="zt5")
            nc.vector.tensor_tensor(out=zt5[:], in0=zt4[:, :, 0:3],
                                    in1=zt4[:, :, 3:6], op=OP.add)
            z_t = big.tile([P, JC], f32, tag="z")
            nc.vector.tensor_tensor(out=z_t[:], in0=zt5[:, :, 0], in1=zt5[:, :, 1], op=OP.add)
            nc.vector.tensor_tensor(out=z_t[:], in0=z_t[:], in1=zt5[:, :, 2], op=OP.add)
            rz = big.tile([P, JC], f32, tag="rz")
            nc.vector.reciprocal(out=rz[:], in_=z_t[:])
            ep = big.tile([P, JC, M], f8e4, tag="ep")
            nc.vector.tensor_tensor(out=ep[:], in0=exp_t[:], in1=bc(rz[:], 2, M), op=OP.mult)
            for j in range(0, JC, 2):
                nc.tensor.matmul(out=C_ps[:], lhsT=t_t[:, j:j + 2, :], rhs=ep[:, j:j + 2, :],
                                 perf_mode=DROW,
                                 start=(c == 0 and j == 0),
                                 stop=(c == NCHUNK - 1 and j == JC - 2))

        Cs = sing.tile([M, M], f32)
        nc.vector.tensor_copy(out=Cs[:], in_=C_ps[:])
        # trace(C) = sum_pix sum_m true_r * portion (the dice numerator / 2);
        # sum of C = sum_pix true_r since each softmax row of ep sums to 1.
        scr_c = sing.tile([M, M], f32)
        nc.vector.tensor_tensor(out=scr_c[:], in0=Cs[:], in1=ident[:], op=OP.mult)
        nc.vector.reduce_sum(out=stats[0:M, 2:3], in_=scr_c[:], axis=AX.X)
        nc.vector.reduce_sum(out=stats[0:M, 3:4], in_=Cs[:], axis=AX.X)

        # ---------- occupancy CE ----------
        occ_t = sing.tile([P, J, K], f8)
        nc.sync.dma_start(out=occ_t[:], in_=occ_sl.ap())
        xsel_t = sing.tile([P, J], bf16)
        nc.sync.dma_start(out=xsel_t[:], in_=xsel_d.ap())
        e4 = sing.tile([P, J, K], bf16)
        nc.scalar.activation(out=e4[:], in_=occ_t[:], func=AF.Exp)
        s4 = sing.tile([P, J], f32)
        nc.vector.reduce_sum(out=s4[:], in_=e4[:], axis=AX.X)
        lse = sing.tile([P, J], f32)
        nc.scalar.activation(out=lse[:], in_=s4[:], func=AF.Ln)
        d4 = sing.tile([P, J], f32)
        nc.vector.tensor_tensor(out=d4[:], in0=lse[:], in1=xsel_t[:], op=OP.subtract)
        nc.vector.reduce_sum(out=stats[:, 4:5], in_=d4[:], axis=AX.X)

        # ---------- class loss (partition 0) ----------
        expc = sing.tile([1, Q], f32)
        nc.scalar.activation(out=expc[:], in_=iel[:], func=AF.Exp)
        sp = sing.tile([1, Q], f32)
        nc.scalar.activation(out=sp[:], in_=expc[:], func=AF.Ln, bias=1.0)
        t9 = sing.tile([1, Q], f32)
        nc.vector.tensor_scalar(out=t9[:], in0=sp[:], scalar1=0.9, scalar2=None, op0=OP.mult)
        nc.vector.tensor_tensor(out=t9[:], in0=t9[:], in1=iel[:], op=OP.subtract)
        scr_q = sing.tile([1, Q], f32)
        nc.vector.tensor_tensor(out=scr_q[:], in0=t9[:], in1=lab[:], op=OP.mult)
        clsm = sing.tile([1, 1], f32)
        nc.vector.reduce_sum(out=clsm[:], in_=scr_q[:], axis=AX.X)
        spsum = sing.tile([1, 1], f32)
        nc.vector.reduce_sum(out=spsum[:], in_=sp[:], axis=AX.X)
        nc.vector.tensor_scalar(out=spsum[:], in0=spsum[:], scalar1=NO_E, scalar2=None, op0=OP.mult)
        nc.vector.tensor_tensor(out=res[:, 6:7], in0=spsum[:], in1=clsm[:], op=OP.add)

        # ---------- NLL (96 partitions, f32 — the only term that needs precision) ----------
        d_ = sing.tile([M, 2], f32)
        nc.vector.tensor_tensor(out=d_[:], in0=ptsr[:], in1=cenr[:], op=OP.subtract)
        r00 = sing.tile([M, 1], f32)
        nc.vector.reciprocal(out=r00[:], in_=cholr[:, 0:1])
        r11 = sing.tile([M, 1], f32)
        nc.vector.reciprocal(out=r11[:], in_=cholr[:, 3:4])
        z0 = sing.tile([M, 1], f32)
        nc.vector.tensor_tensor(out=z0[:], in0=d_[:, 0:1], in1=r00[:], op=OP.mult)
        t1 = sing.tile([M, 1], f32)
        nc.vector.tensor_tensor(out=t1[:], in0=cholr[:, 2:3], in1=z0[:], op=OP.mult)
        nc.vector.tensor_tensor(out=t1[:], in0=d_[:, 1:2], in1=t1[:], op=OP.subtract)
        z1 = sing.tile([M, 1], f32)
        nc.vector.tensor_tensor(out=z1[:], in0=t1[:], in1=r11[:], op=OP.mult)
        sq = sing.tile([M, 1], f32)
        nc.vector.tensor_tensor(out=sq[:], in0=z0[:], in1=z0[:], op=OP.mult)
        sq1 = sing.tile([M, 1], f32)
        nc.vector.tensor_tensor(out=sq1[:], in0=z1[:], in1=z1[:], op=OP.mult)
        nc.vector.tensor_tensor(out=sq[:], in0=sq[:], in1=sq1[:], op=OP.add)
        ldet = sing.tile([M, 1], f32)
        nc.vector.tensor_tensor(out=ldet[:], in0=cholr[:, 0:1], in1=cholr[:, 3:4], op=OP.mult)
        lnd = sing.tile([M, 1], f32)
        nc.scalar.activation(out=lnd[:], in_=ldet[:], func=AF.Ln)
        nc.vector.tensor_scalar(out=sq[:], in0=sq[:], scalar1=0.5,
                                scalar2=float(np.log(2.0 * np.pi)), op0=OP.mult, op1=OP.add)
        nc.vector.tensor_tensor(out=stats[0:M, 0:1], in0=sq[:], in1=lnd[:], op=OP.add)

        # ---------- window extraction + bce ----------
        tv = sing.tile([M, WIN * WIN], f32)
        nc.vector.tensor_copy(out=tv[:].rearrange("m (a b) -> m a b", a=WIN),
                              in_=restride(twr[:], [[W, WIN], [1, WIN]]))
        lg = sing.tile([M, WIN * WIN], f32)
        nc.vector.tensor_copy(out=lg[:].rearrange("m (a b) -> m a b", a=WIN),
                              in_=restride(bwr[:], [[W, WIN], [1, WIN]]))
        expw = sing.tile([M, WIN * WIN], f32)
        nc.scalar.activation(out=expw[:], in_=lg[:], func=AF.Exp)
        spw = sing.tile([M, WIN * WIN], f32)
        nc.scalar.activation(out=spw[:], in_=expw[:], func=AF.Ln, bias=1.0)
        prw = sing.tile([M, WIN * WIN], f32)
        nc.vector.tensor_tensor(out=prw[:], in0=lg[:], in1=tv[:], op=OP.mult)
        nc.vector.tensor_tensor(out=spw[:], in0=spw[:], in1=prw[:], op=OP.subtract)
        valid49 = sing.tile([M, WIN * WIN], f32)
        nc.vector.tensor_copy(out=valid49[:].rearrange("m (a b) -> m a b", a=WIN),
                              in_=bc(valid[:], 2, WIN))
        scr_w = sing.tile([M, WIN * WIN], f32)
        nc.vector.tensor_tensor(out=scr_w[:], in0=spw[:], in1=valid49[:], op=OP.mult)
        nc.vector.reduce_sum(out=stats[0:M, 1:2], in_=scr_w[:], axis=AX.X)

        # ---------- final cross-partition reduction ----------
        fin_ps = ps.tile([1, 6], f32)
        nc.tensor.matmul(out=fin_ps[:], lhsT=ones[:], rhs=stats[:], start=True, stop=True)
        nc.vector.tensor_copy(out=res[:, 0:6], in_=fin_ps[:])
        nc.sync.dma_start(out=partials.ap(), in_=res[:])

    nc.compile()
    return nc


def _get_nc():
    if "nc" not in _CACHE:
        _CACHE["nc"] = _build_nc()
    return _CACHE["nc"]


def make_in_maps(is_electron_logit, true_segmap, binary_mask_logits, portion_logits,
                 incidence_points, positions, chol, occupancy_logits, occupancy_true,
                 matched_q, matched_e):
    import ml_dtypes
    f = np.float32
    f8 = ml_dtypes.float8_e3m4
    bf = ml_dtypes.bfloat16
    dr7 = np.tile(np.arange(WIN, dtype=f), (M, 1))
    mbase = (np.arange(M, dtype=f) * FULLPIX).reshape(M, 1)
    ident = np.eye(M, dtype=f)
    in_maps = []
    for b in range(B):
        me = np.asarray(matched_e[b])
        mq = np.asarray(matched_q[b])
        f8e4 = np.dtype(ml_dtypes.float8_e4m3)
        true_r = np.asarray(true_segmap[b])[:, :, me]          # [H, W, M]
        por_r = np.asarray(portion_logits[b])[:, :, mq]        # [H, W, M]
        bin_r = np.asarray(binary_mask_logits[b])[:, :, mq]    # [H, W, M]
        twin_b = np.ascontiguousarray(true_r.reshape(FULLPIX, M).T).astype(f8)
        bwin_b = np.ascontiguousarray(bin_r.reshape(FULLPIX, M).T).astype(f8)
        ptsr = np.asarray(incidence_points[b], dtype=f)[me]
        cenr = np.asarray(positions[b], dtype=f)[mq]
        cholr = np.asarray(chol[b], dtype=f).reshape(Q, 4)[mq]
        iel = np.asarray(is_electron_logit, dtype=f).reshape(B, Q)[b].reshape(1, Q)
        lab = np.zeros((1, Q), dtype=f)
        lab[0, mq] = 1.0
        occ_b = np.asarray(occupancy_logits[b], dtype=f)
        occt_b = np.asarray(occupancy_true[b])
        xsel_b = np.take_along_axis(occ_b.reshape(FULLPIX, K),
                                    occt_b.reshape(FULLPIX, 1), axis=1)
        for h in range(2):
            sl = slice(h * HALF, (h + 1) * HALF)
            psl = slice(h * NPIX, (h + 1) * NPIX)
            in_maps.append(dict(
                por_sl=np.ascontiguousarray(por_r[sl]).reshape(NPIX, M).astype(f8),
                true_sl=np.ascontiguousarray(true_r[sl]).reshape(NPIX, M).astype(f8e4),
                occ_sl=np.ascontiguousarray(occ_b[sl]).reshape(P, J, K).astype(f8),
                xsel=np.ascontiguousarray(xsel_b[psl]).reshape(P, J).astype(bf),
                twin=twin_b, bwin=bwin_b,
                ptsr=ptsr, cenr=cenr, cholr=cholr, iel=iel, lab=lab,
                dr7=dr7,
                wlo=np.full((M, 1), float(h * HALF), dtype=f),
                whi=np.full((M, 1), float(h * HALF + HALF - 1), dtype=f),
                mbase=mbase, ident=ident,
            ))
    return in_maps


def combine(partials_list):
    s = np.stack([np.asarray(p, dtype=np.float64).reshape(8) for p in partials_list])
    # slots: 0=nll_sum 1=bce_sum 2=trace(C) 3=sum(C)=sum_true 4=occ_sum 6=class_sum
    class_loss = s[0::2, 6].sum() / (B * Q)
    nll_loss = s[0::2, 0].sum() / (B * M)
    bce_loss = s[:, 1].sum() / (B * M * WIN * WIN)
    occ_loss = s[:, 4].sum() / (B * H * W)
    dice = 0.0
    for b in range(B):
        num = 2.0 * (s[2 * b, 2] + s[2 * b + 1, 2])
        den = s[2 * b, 3] + s[2 * b + 1, 3] + H * W
        dice += 1.0 - (num + 1.0) / (den + 1.0)
    dice_loss = dice / B
    return np.float32(class_loss + bce_loss + dice_loss + nll_loss + occ_loss)


def kernel(**inputs):
    from concourse.bass_utils import run_bass_kernel_spmd
    nc = _get_nc()
    in_maps = make_in_maps(**{k: np.asarray(v) for k, v in inputs.items()})
    r = run_bass_kernel_spmd(nc, in_maps, list(range(8)))
    return combine([r.results[c]["partials"] for c in range(8)])


# revision 15
# speedup vs baseline: 2.1700x; 1.1115x over previous
"""Trainium2 Bass kernel for nn_Criterion_32830730011569.

Strategy: 8 cores = (image b in 0..3) x (H-half h in 0..1). The host
pre-gathers the matched channels (true_r = true[..., me], por_r = por[..., mq])
and ships the per-core pixel slices as fp8(e3m4), so each core streams only
[18432, 96] x 2 plus the tiny occupancy tensors (~3.9MB vs 19MB of f32).

Dice per chunk of 24 pixel-rows: ACT computes exp(por_r) writing a transposed
[P, m, j] tile so the per-pixel 1/Z product runs in the DVE 2x bf16 mode with
the [P, j] reciprocal broadcast across the middle (m) axis. PE accumulates
C[m_t, m_e] = sum_pix true_r * softmax over one PSUM bank; trace(C) is the
dice numerator and the full sum of C is sum(true_r) (softmax rows sum to 1),
giving the denominator for free.

The 7x7-window BCE gathers one contiguous 1159-pixel run per matched electron
from host-built channel-major [M, H*W] images (222KB total vs 4.1MB), then
extracts the 7x7 with a strided copy. Occupancy CE streams fp8 logits plus the
host-pre-gathered label logit. NLL/class stay f32 on pre-gathered small
tensors. ln/exp both come from the natural_log_exp ACT table set (one load).
Each core returns 7 partial sums; the host combines them into the scalar loss.
"""
import sys

sys.path.insert(0, "/opt/trn_rl_repo")
import numpy as np

B, H, W, Q, E, M, K, WIN = 4, 192, 192, 160, 96, 96, 4, 7
NO_E = 0.1
HALF = H // 2          # rows per core slice
NPIX = HALF * W        # 18432 pixels per slice
FULLPIX = H * W        # 36864 pixels per image
P = 128                # partitions
J = NPIX // P          # 144 pixels per partition (p-major)
JC = 36                # pixels per chunk per partition
NCHUNK = J // JC       # 4
RUN = (WIN - 1) * W + WIN  # 1159: contiguous window-row span
MAGIC = 8388608.0      # 2^23

_CACHE = {}


def _build_nc():
    import concourse.bass as bass
    import concourse.bacc as bacc
    import concourse.tile as tile
    from concourse import mybir

    f32 = mybir.dt.float32
    i32 = mybir.dt.int32
    bf16 = mybir.dt.bfloat16
    f8 = mybir.dt.float8e3
    f8e4 = mybir.dt.float8e4
    AF = mybir.ActivationFunctionType
    OP = mybir.AluOpType
    AX = mybir.AxisListType
    DROW = mybir.MatmulPerfMode.DoubleRow

    nc = bacc.Bacc("TRN2", target_bir_lowering=False, debug=False, num_devices=8)

    # ---- external I/O ----
    por_sl = nc.dram_tensor("por_sl", [NPIX, M], f8, kind="ExternalInput")
    true_sl = nc.dram_tensor("true_sl", [NPIX, M], f8e4, kind="ExternalInput")
    occ_sl = nc.dram_tensor("occ_sl", [P, J, K], f8, kind="ExternalInput")
    xsel_d = nc.dram_tensor("xsel", [P, J], bf16, kind="ExternalInput")
    twin = nc.dram_tensor("twin", [M, FULLPIX], f8, kind="ExternalInput")
    bwin = nc.dram_tensor("bwin", [M, FULLPIX], f8, kind="ExternalInput")
    ptsr_d = nc.dram_tensor("ptsr", [M, 2], f32, kind="ExternalInput")
    cenr_d = nc.dram_tensor("cenr", [M, 2], f32, kind="ExternalInput")
    cholr_d = nc.dram_tensor("cholr", [M, 4], f32, kind="ExternalInput")
    iel_d = nc.dram_tensor("iel", [32, 5], f32, kind="ExternalInput")
    lab_d = nc.dram_tensor("lab", [32, 5], f32, kind="ExternalInput")
    dr7_d = nc.dram_tensor("dr7", [M, WIN], f32, kind="ExternalInput")
    wlo_d = nc.dram_tensor("wlo", [M, 1], f32, kind="ExternalInput")
    whi_d = nc.dram_tensor("whi", [M, 1], f32, kind="ExternalInput")
    mbase_d = nc.dram_tensor("mbase", [M, 1], f32, kind="ExternalInput")
    ident_d = nc.dram_tensor("ident", [M, M], f32, kind="ExternalInput")
    partials = nc.dram_tensor("partials", [1, 8], f32, kind="ExternalOutput")

    def bc(ap, pos, count):
        """Insert a stride-0 broadcast dim into an AP at free-dim position pos."""
        new = list(ap.ap)
        new.insert(pos, [0, count])
        return bass.AP(tensor=ap.tensor, offset=ap.offset, ap=new)

    def restride(ap, dims):
        """Replace the free dims of a 2D AP with explicit [step, count] pairs."""
        new_ap = [ap.ap[0]] + [list(d) for d in dims]
        return bass.AP(tensor=ap.tensor, offset=ap.offset, ap=new_ap)

    from contextlib import ExitStack

    with tile.TileContext(nc) as tc, ExitStack() as ctx:
        sing = ctx.enter_context(tc.tile_pool(name="sing", bufs=1))
        big = ctx.enter_context(tc.tile_pool(name="big", bufs=2))
        ps = ctx.enter_context(tc.tile_pool(name="ps", bufs=1, space="PSUM"))

        # small loads go on the gpsimd queue so the sync queue dispatches
        # the first dice chunk DMAs immediately
        def load(dram, shape, dt=f32):
            nm = dram.name + "_sb"
            t = sing.tile(shape, dt, name=nm, tag=nm)
            nc.gpsimd.dma_start(out=t[:], in_=dram.ap())
            return t

        ones = sing.tile([P, 1], f32)
        nc.vector.memset(ones[:], 1.0)
        stats = sing.tile([P, 8], f32)
        nc.vector.memset(stats[:], 0.0)
        res = sing.tile([1, 8], f32)
        nc.vector.memset(res[:], 0.0)

        ptsr = load(ptsr_d, [M, 2])
        cenr = load(cenr_d, [M, 2])
        cholr = load(cholr_d, [M, 4])
        iel = load(iel_d, [32, 5])
        lab = load(lab_d, [32, 5])
        dr7 = load(dr7_d, [M, WIN])
        wlo = load(wlo_d, [M, 1])
        whi = load(whi_d, [M, 1])
        mbase = load(mbase_d, [M, 1])
        ident = load(ident_d, [M, M])
        occ_t = sing.tile([P, J, K], f8)
        nc.gpsimd.dma_start(out=occ_t[:], in_=occ_sl.ap())
        xsel_t = sing.tile([P, J], bf16)
        nc.gpsimd.dma_start(out=xsel_t[:], in_=xsel_d.ap())

        # ---------- dice streaming ----------
        por_v = por_sl.ap().rearrange("(p j) m -> p j m", p=P)
        true_v = true_sl.ap().rearrange("(p j) m -> p j m", p=P)
        C_ps = ps.tile([M, M], f32)

        def dice_chunk(c):
            sl = slice(c * JC, (c + 1) * JC)
            por_t = big.tile([P, JC, M], f8, tag="por")
            nc.sync.dma_start(out=por_t[:], in_=por_v[:, sl, :])
            t_t = big.tile([P, JC, M], f8e4, tag="t")
            nc.sync.dma_start(out=t_t[:], in_=true_v[:, sl, :])
            exp_t = big.tile([P, JC, M], bf16, tag="exp")
            nc.scalar.activation(out=exp_t[:], in_=por_t[:], func=AF.Exp)
            # Z via pairwise-tree adds (DVE 2x bf16 mode) + one small reduce;
            # a tensor_reduce over the full 96 would run 1x
            zt1 = big.tile([P, JC, 48], bf16, tag="zt1")
            nc.vector.tensor_tensor(out=zt1[:], in0=exp_t[:, :, 0:48],
                                    in1=exp_t[:, :, 48:96], op=OP.add)
            zt2 = big.tile([P, JC, 24], bf16, tag="zt2")
            nc.vector.tensor_tensor(out=zt2[:], in0=zt1[:, :, 0:24],
                                    in1=zt1[:, :, 24:48], op=OP.add)
            zt3 = big.tile([P, JC, 12], bf16, tag="zt3")
            nc.vector.tensor_tensor(out=zt3[:], in0=zt2[:, :, 0:12],
                                    in1=zt2[:, :, 12:24], op=OP.add)
            zt4 = big.tile([P, JC, 6], bf16, tag="zt4")
            nc.vector.tensor_tensor(out=zt4[:], in0=zt3[:, :, 0:6],
                                    in1=zt3[:, :, 6:12], op=OP.add)
            z_t = big.tile([P, JC], f32, tag="z")
            nc.vector.reduce_sum(out=z_t[:], in_=zt4[:], axis=AX.X)
            rz = big.tile([P, JC], f32, tag="rz")
            nc.vector.reciprocal(out=rz[:], in_=z_t[:])
            # replicate 1/Z into 4 contiguous bf16 lanes so the softmax
            # product below has a unit-stride 16-bit in1 -> DVE 2x mode
            rz4 = big.tile([P, JC, 4], bf16, tag="rz4")
            nc.vector.tensor_scalar(out=rz4[:], in0=bc(rz[:], 2, 4), scalar1=1.0,
                                    scalar2=None, op0=OP.mult)
            ep = big.tile([P, JC, M], bf16, tag="ep")
            nc.vector.tensor_tensor(
                out=ep[:].rearrange("p j (g i) -> p j g i", i=4),
                in0=exp_t[:].rearrange("p j (g i) -> p j g i", i=4),
                in1=bc(rz4[:], 2, M // 4), op=OP.mult)
            for j in range(JC):
                nc.tensor.matmul(out=C_ps[:], lhsT=t_t[:, j, :], rhs=ep[:, j, :],
                                 start=(c == 0 and j == 0),
                                 stop=(c == NCHUNK - 1 and j == JC - 1))

        dice_chunk(0)

        # ---------- occupancy exp (fills the chunk-0 DMA wait) ----------
        e4 = sing.tile([P, J, K], bf16)
        nc.scalar.activation(out=e4[:], in_=occ_t[:], func=AF.Exp)

        dice_chunk(1)

        # ---------- window offsets (floor via 2^23 magic) ----------
        rmag = sing.tile([M, 2], f32)
        nc.vector.tensor_scalar(out=rmag[:], in0=ptsr[:], scalar1=MAGIC, scalar2=-MAGIC,
                                op0=OP.add, op1=OP.add)
        gtm = sing.tile([M, 2], f32)
        nc.vector.tensor_tensor(out=gtm[:], in0=rmag[:], in1=ptsr[:], op=OP.is_gt)
        pixf = sing.tile([M, 2], f32)
        nc.vector.tensor_tensor(out=pixf[:], in0=rmag[:], in1=gtm[:], op=OP.subtract)
        # run start in full-image flat pixels: (r-3)*W + (c-3); always in
        # bounds because incidence points live in [4, 187]
        base = sing.tile([M, 1], f32)
        nc.vector.tensor_scalar(out=base[:], in0=pixf[:, 0:1], scalar1=float(W),
                                scalar2=float(-3 * W - 3), op0=OP.mult, op1=OP.add)
        nc.vector.tensor_tensor(out=base[:], in0=base[:], in1=pixf[:, 1:2], op=OP.add)
        soff = sing.tile([M, 1], f32)
        nc.vector.tensor_scalar(out=soff[:], in0=base[:], scalar1=mbase[:], scalar2=MAGIC,
                                op0=OP.add, op1=OP.add)
        soff_i = sing.tile([M, 1], i32)
        nc.vector.tensor_scalar(out=soff_i[:], in0=soff[:].bitcast(i32), scalar1=0x007FFFFF,
                                scalar2=None, op0=OP.bitwise_and)
        # per-window-row validity: absolute row r-3+dr inside this core's half
        p0m3 = sing.tile([M, 1], f32)
        nc.vector.tensor_scalar(out=p0m3[:], in0=pixf[:, 0:1], scalar1=-3.0, scalar2=None,
                                op0=OP.add)
        rows7 = sing.tile([M, WIN], f32)
        nc.gpsimd.tensor_scalar(out=rows7[:], in0=dr7[:], scalar1=p0m3[:], scalar2=None,
                                op0=OP.add)
        v1 = sing.tile([M, WIN], f32)
        nc.gpsimd.tensor_scalar(out=v1[:], in0=rows7[:], scalar1=wlo[:], scalar2=None, op0=OP.is_ge)
        v2 = sing.tile([M, WIN], f32)
        nc.gpsimd.tensor_scalar(out=v2[:], in0=rows7[:], scalar1=whi[:], scalar2=None, op0=OP.is_le)
        valid = sing.tile([M, WIN], f32)
        nc.gpsimd.tensor_tensor(out=valid[:], in0=v1[:], in1=v2[:], op=OP.mult)

        # ---------- window gathers: one 1159-element run per matched electron ----------
        twr = sing.tile([M, RUN], f8)
        bwr = sing.tile([M, RUN], f8)
        twin_flat = bass.AP(tensor=twin.ap().tensor, offset=0, ap=[[1, M * FULLPIX], [1, 1]])
        bwin_flat = bass.AP(tensor=bwin.ap().tensor, offset=0, ap=[[1, M * FULLPIX], [1, 1]])
        nc.gpsimd.indirect_dma_start(
            out=twr[:], out_offset=None, in_=twin_flat,
            in_offset=bass.IndirectOffsetOnAxis(ap=soff_i[:, 0:1], axis=0))
        nc.gpsimd.indirect_dma_start(
            out=bwr[:], out_offset=None, in_=bwin_flat,
            in_offset=bass.IndirectOffsetOnAxis(ap=soff_i[:, 0:1], axis=0))

        dice_chunk(2)
        dice_chunk(3)

        # ---------- window extraction + exp, occ s4, class exp ----------
        expc = sing.tile([32, 5], f32)
        nc.scalar.activation(out=expc[:], in_=iel[:], func=AF.Exp)
        tv = sing.tile([M, WIN * WIN], f32)
        nc.vector.tensor_copy(out=tv[:].rearrange("m (a b) -> m a b", a=WIN),
                              in_=restride(twr[:], [[W, WIN], [1, WIN]]))
        lg = sing.tile([M, WIN * WIN], f32)
        nc.vector.tensor_copy(out=lg[:].rearrange("m (a b) -> m a b", a=WIN),
                              in_=restride(bwr[:], [[W, WIN], [1, WIN]]))
        expw = sing.tile([M, WIN * WIN], f32)
        nc.scalar.activation(out=expw[:], in_=lg[:], func=AF.Exp)
        s4 = sing.tile([P, J], f32)
        nc.vector.reduce_sum(out=s4[:], in_=e4[:], axis=AX.X)

        # ---------- NLL prelude (f32 - the only term that needs precision) ----------
        d_ = sing.tile([M, 2], f32)
        nc.vector.tensor_tensor(out=d_[:], in0=ptsr[:], in1=cenr[:], op=OP.subtract)
        r00 = sing.tile([M, 1], f32)
        nc.vector.reciprocal(out=r00[:], in_=cholr[:, 0:1])
        r11 = sing.tile([M, 1], f32)
        nc.vector.reciprocal(out=r11[:], in_=cholr[:, 3:4])
        z0 = sing.tile([M, 1], f32)
        nc.vector.tensor_tensor(out=z0[:], in0=d_[:, 0:1], in1=r00[:], op=OP.mult)
        t1 = sing.tile([M, 1], f32)
        nc.vector.tensor_tensor(out=t1[:], in0=cholr[:, 2:3], in1=z0[:], op=OP.mult)
        nc.vector.tensor_tensor(out=t1[:], in0=d_[:, 1:2], in1=t1[:], op=OP.subtract)
        z1 = sing.tile([M, 1], f32)
        nc.vector.tensor_tensor(out=z1[:], in0=t1[:], in1=r11[:], op=OP.mult)
        sq = sing.tile([M, 1], f32)
        nc.vector.tensor_tensor(out=sq[:], in0=z0[:], in1=z0[:], op=OP.mult)
        sq1 = sing.tile([M, 1], f32)
        nc.vector.tensor_tensor(out=sq1[:], in0=z1[:], in1=z1[:], op=OP.mult)
        nc.vector.tensor_tensor(out=sq[:], in0=sq[:], in1=sq1[:], op=OP.add)
        ldet = sing.tile([M, 1], f32)
        nc.vector.tensor_tensor(out=ldet[:], in0=cholr[:, 0:1], in1=cholr[:, 3:4], op=OP.mult)

        # ---------- Ln cluster (single ACT table switch) ----------
        lse = sing.tile([P, J], f32)
        nc.scalar.activation(out=lse[:], in_=s4[:], func=AF.Ln)
        sp = sing.tile([32, 5], f32)
        nc.scalar.activation(out=sp[:], in_=expc[:], func=AF.Ln, bias=1.0)
        lnd = sing.tile([M, 1], f32)
        nc.scalar.activation(out=lnd[:], in_=ldet[:], func=AF.Ln)
        spw = sing.tile([M, WIN * WIN], f32)
        nc.scalar.activation(out=spw[:], in_=expw[:], func=AF.Ln, bias=1.0)

        # ---------- finishers ----------
        # occupancy CE
        d4 = sing.tile([P, J], f32)
        nc.gpsimd.tensor_tensor(out=d4[:], in0=lse[:], in1=xsel_t[:], op=OP.subtract)
        nc.vector.reduce_sum(out=stats[:, 4:5], in_=d4[:], axis=AX.X)
        # class loss (32 partitions, folded via the final ones-matmul)
        t9 = sing.tile([32, 5], f32)
        nc.gpsimd.tensor_scalar(out=t9[:], in0=sp[:], scalar1=0.9, scalar2=None, op0=OP.mult)
        nc.gpsimd.tensor_tensor(out=t9[:], in0=t9[:], in1=iel[:], op=OP.subtract)
        nc.gpsimd.tensor_tensor(out=t9[:], in0=t9[:], in1=lab[:], op=OP.mult)
        nc.vector.reduce_sum(out=stats[0:32, 6:7], in_=t9[:], axis=AX.X)
        nc.vector.reduce_sum(out=stats[0:32, 5:6], in_=sp[:], axis=AX.X)
        # nll
        nc.vector.tensor_scalar(out=sq[:], in0=sq[:], scalar1=0.5,
                                scalar2=float(np.log(2.0 * np.pi)), op0=OP.mult, op1=OP.add)
        nc.vector.tensor_tensor(out=stats[0:M, 0:1], in0=sq[:], in1=lnd[:], op=OP.add)
        # window bce
        prw = sing.tile([M, WIN * WIN], f32)
        nc.gpsimd.tensor_tensor(out=prw[:], in0=lg[:], in1=tv[:], op=OP.mult)
        nc.gpsimd.tensor_tensor(out=prw[:], in0=spw[:], in1=prw[:], op=OP.subtract)
        valid49 = sing.tile([M, WIN * WIN], f32)
        nc.gpsimd.tensor_copy(out=valid49[:].rearrange("m (a b) -> m a b", a=WIN),
                              in_=bc(valid[:], 2, WIN))
        scr_w = sing.tile([M, WIN * WIN], f32)
        nc.gpsimd.tensor_tensor(out=scr_w[:], in0=prw[:], in1=valid49[:], op=OP.mult)
        nc.vector.reduce_sum(out=stats[0:M, 1:2], in_=scr_w[:], axis=AX.X)
        # dice: trace(C) and sum(C)
        Cs = sing.tile([M, M], f32)
        nc.vector.tensor_copy(out=Cs[:], in_=C_ps[:])
        scr_c = sing.tile([M, M], f32)
        nc.gpsimd.tensor_tensor(out=scr_c[:], in0=Cs[:], in1=ident[:], op=OP.mult)
        nc.vector.reduce_sum(out=stats[0:M, 2:3], in_=scr_c[:], axis=AX.X)
        nc.vector.reduce_sum(out=stats[0:M, 3:4], in_=Cs[:], axis=AX.X)

        # ---------- final cross-partition reduction ----------
        fin_ps = ps.tile([1, 8], f32)
        nc.tensor.matmul(out=fin_ps[:], lhsT=ones[:], rhs=stats[:], start=True, stop=True)
        nc.vector.tensor_copy(out=res[:, 0:8], in_=fin_ps[:])
        nc.sync.dma_start(out=partials.ap(), in_=res[:])

    nc.compile()
    return nc


def _get_nc():
    if "nc" not in _CACHE:
        _CACHE["nc"] = _build_nc()
    return _CACHE["nc"]


def make_in_maps(is_electron_logit, true_segmap, binary_mask_logits, portion_logits,
                 incidence_points, positions, chol, occupancy_logits, occupancy_true,
                 matched_q, matched_e):
    import ml_dtypes
    f = np.float32
    f8 = ml_dtypes.float8_e3m4
    bf = ml_dtypes.bfloat16
    dr7 = np.tile(np.arange(WIN, dtype=f), (M, 1))
    mbase = (np.arange(M, dtype=f) * FULLPIX).reshape(M, 1)
    ident = np.eye(M, dtype=f)
    in_maps = []
    for b in range(B):
        me = np.asarray(matched_e[b])
        mq = np.asarray(matched_q[b])
        f8e4 = np.dtype(ml_dtypes.float8_e4m3)
        true_r = np.asarray(true_segmap[b])[:, :, me]          # [H, W, M]
        por_r = np.asarray(portion_logits[b])[:, :, mq]        # [H, W, M]
        bin_r = np.asarray(binary_mask_logits[b])[:, :, mq]    # [H, W, M]
        twin_b = np.ascontiguousarray(true_r.reshape(FULLPIX, M).T).astype(f8)
        bwin_b = np.ascontiguousarray(bin_r.reshape(FULLPIX, M).T).astype(f8)
        ptsr = np.asarray(incidence_points[b], dtype=f)[me]
        cenr = np.asarray(positions[b], dtype=f)[mq]
        cholr = np.asarray(chol[b], dtype=f).reshape(Q, 4)[mq]
        iel = np.asarray(is_electron_logit, dtype=f).reshape(B, Q)[b].reshape(32, 5)
        lab = np.zeros(Q, dtype=f)
        lab[mq] = 1.0
        lab = lab.reshape(32, 5)
        occ_b = np.asarray(occupancy_logits[b], dtype=f)
        occt_b = np.asarray(occupancy_true[b])
        xsel_b = np.take_along_axis(occ_b.reshape(FULLPIX, K),
                                    occt_b.reshape(FULLPIX, 1), axis=1)
        for h in range(2):
            sl = slice(h * HALF, (h + 1) * HALF)
            psl = slice(h * NPIX, (h + 1) * NPIX)
            in_maps.append(dict(
                por_sl=np.ascontiguousarray(por_r[sl]).reshape(NPIX, M).astype(f8),
                true_sl=np.ascontiguousarray(true_r[sl]).reshape(NPIX, M).astype(f8e4),
                occ_sl=np.ascontiguousarray(occ_b[sl]).reshape(P, J, K).astype(f8),
                xsel=np.ascontiguousarray(xsel_b[psl]).reshape(P, J).astype(bf),
                twin=twin_b, bwin=bwin_b,
                ptsr=ptsr, cenr=cenr, cholr=cholr, iel=iel, lab=lab,
                dr7=dr7,
                wlo=np.full((M, 1), float(h * HALF), dtype=f),
                whi=np.full((M, 1), float(h * HALF + HALF - 1), dtype=f),
                mbase=mbase, ident=ident,
            ))
    return in_maps


def combine(partials_list):
    s = np.stack([np.asarray(p, dtype=np.float64).reshape(8) for p in partials_list])
    # slots: 0=nll_sum 1=bce_sum 2=trace(C) 3=sum(C)=sum_true 4=occ_sum
    # 5=softplus_sum 6=matched(0.9*sp - x) sum
    class_loss = (NO_E * s[0::2, 5].sum() + s[0::2, 6].sum()) / (B * Q)
    nll_loss = s[0::2, 0].sum() / (B * M)
    bce_loss = s[:, 1].sum() / (B * M * WIN * WIN)
    occ_loss = s[:, 4].sum() / (B * H * W)
    dice = 0.0
    for b in range(B):
        num = 2.0 * (s[2 * b, 2] + s[2 * b + 1, 2])
        den = s[2 * b, 3] + s[2 * b + 1, 3] + H * W
        dice += 1.0 - (num + 1.0) / (den + 1.0)
    dice_loss = dice / B
    return np.float32(class_loss + bce_loss + dice_loss + nll_loss + occ_loss)


def kernel(**inputs):
    from concourse.bass_utils import run_bass_kernel_spmd
    nc = _get_nc()
    in_maps = make_in_maps(**{k: np.asarray(v) for k, v in inputs.items()})
    r = run_bass_kernel_spmd(nc, in_maps, list(range(8)))
    return combine([r.results[c]["partials"] for c in range(8)])


# revision 18
# speedup vs baseline: 2.6109x; 1.2032x over previous
"""Trainium2 Bass kernel for nn_Criterion_32830730011569.

Strategy: 8 cores = (image b in 0..3) x (H-half h in 0..1). The host
pre-gathers the matched channels (true_r = true[..., me], por_r = por[..., mq])
and ships the per-core pixel slices as fp8, so each core streams only
[18432, 96] x 2 plus the tiny occupancy tensors (~3.9MB vs 19MB of f32).

Dice per chunk of 36 pixel-rows: ACT computes exp(por_r); Z comes from a
pairwise-add tree (DVE 2x bf16 mode, vs always-1x tensor_reduce); the softmax
ep = exp * (1/Z) also runs 2x by replicating 1/Z into 4 contiguous bf16 lanes
so in1 has a unit-stride innermost dim. PE accumulates C[m_t, m_e] =
sum_pix true_r x softmax into one PSUM bank; trace(C) is the dice numerator
and sum(C) = sum(true_r) (softmax rows sum to 1) gives the denominator free.

The 7x7-window BCE gathers one contiguous 1159-pixel run per matched electron
from host-built channel-major [M, H*W] images (222KB vs 4.1MB), then extracts
the 7x7 with a strided copy. Occupancy CE streams fp8 logits plus the host
pre-gathered label logit. NLL/class stay f32 on pre-gathered small tensors.

Scheduling: ln/exp share one ACT table set, and every Ln is pinned (add_dep)
after the last dice exp so the set switches exactly once; gather-dependent
extraction and all tail DVE work are pinned after the last dice softmax so
they cannot head-of-line block the dice pipeline. Small inputs ride one
concatenated DMA on the gpsimd queue, keeping the sync queue free to dispatch
chunk DMAs immediately. Each core returns 8 partial sums; the host combines.
"""
import sys

sys.path.insert(0, "/opt/trn_rl_repo")
import numpy as np

B, H, W, Q, E, M, K, WIN = 4, 192, 192, 160, 96, 96, 4, 7
NO_E = 0.1
HALF = H // 2          # rows per core slice
NPIX = HALF * W        # 18432 pixels per slice
FULLPIX = H * W        # 36864 pixels per image
P = 128                # partitions
J = NPIX // P          # 144 pixels per partition (p-major)
JC = 36                # pixels per chunk per partition
NCHUNK = J // JC       # 4
RUN = (WIN - 1) * W + WIN  # 1159: contiguous window-row span
MAGIC = 8388608.0      # 2^23
NSM = 28               # used columns in the small-constant concat

_CACHE = {}


def _build_nc():
    import concourse.bass as bass
    import concourse.bacc as bacc
    import concourse.tile as tile
    from concourse.tile import add_dep_helper
    from concourse import mybir

    f32 = mybir.dt.float32
    i32 = mybir.dt.int32
    bf16 = mybir.dt.bfloat16
    f8 = mybir.dt.float8e3
    f8e4 = mybir.dt.float8e4
    AF = mybir.ActivationFunctionType
    OP = mybir.AluOpType
    AX = mybir.AxisListType

    nc = bacc.Bacc("TRN2", target_bir_lowering=False, debug=False, num_devices=8)

    # ---- external I/O ----
    por_sl = nc.dram_tensor("por_sl", [NPIX, M], f8, kind="ExternalInput")
    true_sl = nc.dram_tensor("true_sl", [NPIX, M], f8e4, kind="ExternalInput")
    occ_sl = nc.dram_tensor("occ_sl", [P, J, K], f8, kind="ExternalInput")
    xsel_d = nc.dram_tensor("xsel", [P, J], bf16, kind="ExternalInput")
    twin = nc.dram_tensor("twin", [M, FULLPIX], f8, kind="ExternalInput")
    bwin = nc.dram_tensor("bwin", [M, FULLPIX], f8, kind="ExternalInput")
    smalls_d = nc.dram_tensor("smalls", [P, NSM], f32, kind="ExternalInput")
    ident_d = nc.dram_tensor("ident", [M, M], f32, kind="ExternalInput")
    partials = nc.dram_tensor("partials", [1, 8], f32, kind="ExternalOutput")

    def bc(ap, pos, count):
        """Insert a stride-0 broadcast dim into an AP at free-dim position pos."""
        new = list(ap.ap)
        new.insert(pos, [0, count])
        return bass.AP(tensor=ap.tensor, offset=ap.offset, ap=new)

    def restride(ap, dims):
        """Replace the free dims of a 2D AP with explicit [step, count] pairs."""
        new_ap = [ap.ap[0]] + [list(d) for d in dims]
        return bass.AP(tensor=ap.tensor, offset=ap.offset, ap=new_ap)

    from contextlib import ExitStack

    with tile.TileContext(nc) as tc, ExitStack() as ctx:
        sing = ctx.enter_context(tc.tile_pool(name="sing", bufs=1))
        big = ctx.enter_context(tc.tile_pool(name="big", bufs=2))
        ps = ctx.enter_context(tc.tile_pool(name="ps", bufs=1, space="PSUM"))

        ones = sing.tile([P, 1], f32)
        nc.vector.memset(ones[:], 1.0)
        stats = sing.tile([P, 8], f32)
        nc.vector.memset(stats[:], 0.0)
        res = sing.tile([1, 8], f32)
        nc.vector.memset(res[:], 0.0)

        # one concatenated small-constant load on the gpsimd queue; the sync
        # queue stays free to dispatch the dice chunk DMAs immediately
        sm = sing.tile([P, NSM], f32)
        nc.gpsimd.dma_start(out=sm[:], in_=smalls_d.ap())
        ptsr = sm[0:M, 0:2]
        cenr = sm[0:M, 2:4]
        cholr = sm[0:M, 4:8]
        dr7 = sm[0:M, 8:15]
        wlo = sm[0:M, 15:16]
        whi = sm[0:M, 16:17]
        mbase = sm[0:M, 17:18]
        iel = sm[0:32, 18:23]
        lab = sm[0:32, 23:28]
        occ_t = sing.tile([P, J, K], f8)
        nc.gpsimd.dma_start(out=occ_t[:], in_=occ_sl.ap())
        xsel_t = sing.tile([P, J], bf16)
        nc.gpsimd.dma_start(out=xsel_t[:], in_=xsel_d.ap())

        # ---------- window offsets (floor via 2^23 magic), feeds the gathers ----------
        rmag = sing.tile([M, 2], f32)
        nc.vector.tensor_scalar(out=rmag[:], in0=ptsr, scalar1=MAGIC, scalar2=-MAGIC,
                                op0=OP.add, op1=OP.add)
        gtm = sing.tile([M, 2], f32)
        nc.vector.tensor_tensor(out=gtm[:], in0=rmag[:], in1=ptsr, op=OP.is_gt)
        pixf = sing.tile([M, 2], f32)
        nc.vector.tensor_tensor(out=pixf[:], in0=rmag[:], in1=gtm[:], op=OP.subtract)
        # run start in full-image flat pixels: (r-3)*W + (c-3); always in
        # bounds because incidence points live in [4, 187]
        base = sing.tile([M, 1], f32)
        nc.vector.tensor_scalar(out=base[:], in0=pixf[:, 0:1], scalar1=float(W),
                                scalar2=float(-3 * W - 3), op0=OP.mult, op1=OP.add)
        nc.vector.tensor_tensor(out=base[:], in0=base[:], in1=pixf[:, 1:2], op=OP.add)
        soff = sing.tile([M, 1], f32)
        nc.vector.tensor_scalar(out=soff[:], in0=base[:], scalar1=mbase, scalar2=MAGIC,
                                op0=OP.add, op1=OP.add)
        soff_i = sing.tile([M, 1], i32)
        nc.vector.tensor_scalar(out=soff_i[:], in0=soff[:].bitcast(i32), scalar1=0x007FFFFF,
                                scalar2=None, op0=OP.bitwise_and)
        p0m3 = sing.tile([M, 1], f32)
        nc.vector.tensor_scalar(out=p0m3[:], in0=pixf[:, 0:1], scalar1=-3.0, scalar2=None,
                                op0=OP.add)

        # ---------- window gathers: one 1159-element run per matched electron ----------
        twr = sing.tile([M, RUN], f8)
        bwr = sing.tile([M, RUN], f8)
        twin_flat = bass.AP(tensor=twin.ap().tensor, offset=0, ap=[[1, M * FULLPIX], [1, 1]])
        bwin_flat = bass.AP(tensor=bwin.ap().tensor, offset=0, ap=[[1, M * FULLPIX], [1, 1]])
        nc.gpsimd.indirect_dma_start(
            out=twr[:], out_offset=None, in_=twin_flat,
            in_offset=bass.IndirectOffsetOnAxis(ap=soff_i[:, 0:1], axis=0))
        nc.gpsimd.indirect_dma_start(
            out=bwr[:], out_offset=None, in_=bwin_flat,
            in_offset=bass.IndirectOffsetOnAxis(ap=soff_i[:, 0:1], axis=0))
        # per-window-row validity: absolute row r-3+dr inside this core's half
        rows7 = sing.tile([M, WIN], f32)
        nc.gpsimd.tensor_scalar(out=rows7[:], in0=dr7, scalar1=p0m3[:], scalar2=None,
                                op0=OP.add)
        v1 = sing.tile([M, WIN], f32)
        nc.gpsimd.tensor_scalar(out=v1[:], in0=rows7[:], scalar1=wlo, scalar2=None, op0=OP.is_ge)
        v2 = sing.tile([M, WIN], f32)
        nc.gpsimd.tensor_scalar(out=v2[:], in0=rows7[:], scalar1=whi, scalar2=None, op0=OP.is_le)
        valid = sing.tile([M, WIN], f32)
        nc.gpsimd.tensor_tensor(out=valid[:], in0=v1[:], in1=v2[:], op=OP.mult)
        ident = sing.tile([M, M], f32)
        nc.gpsimd.dma_start(out=ident[:], in_=ident_d.ap())

        # ---------- dice streaming ----------
        por_v = por_sl.ap().rearrange("(p j) m -> p j m", p=P)
        true_v = true_sl.ap().rearrange("(p j) m -> p j m", p=P)
        C_ps = ps.tile([M, M], f32)

        def dice_chunk(c):
            sl = slice(c * JC, (c + 1) * JC)
            por_t = big.tile([P, JC, M], f8, tag="por")
            nc.sync.dma_start(out=por_t[:], in_=por_v[:, sl, :])
            t_t = big.tile([P, JC, M], f8e4, tag="t")
            nc.sync.dma_start(out=t_t[:], in_=true_v[:, sl, :])
            exp_t = big.tile([P, JC, M], bf16, tag="exp")
            exp_i = nc.scalar.activation(out=exp_t[:], in_=por_t[:], func=AF.Exp)
            # Z via pairwise-tree adds (DVE 2x bf16 mode) + one small reduce;
            # a tensor_reduce over the full 96 would run 1x
            zt1 = big.tile([P, JC, 48], bf16, tag="zt1")
            nc.vector.tensor_tensor(out=zt1[:], in0=exp_t[:, :, 0:48],
                                    in1=exp_t[:, :, 48:96], op=OP.add)
            zt2 = big.tile([P, JC, 24], bf16, tag="zt2")
            nc.vector.tensor_tensor(out=zt2[:], in0=zt1[:, :, 0:24],
                                    in1=zt1[:, :, 24:48], op=OP.add)
            zt3 = big.tile([P, JC, 12], bf16, tag="zt3")
            nc.vector.tensor_tensor(out=zt3[:], in0=zt2[:, :, 0:12],
                                    in1=zt2[:, :, 12:24], op=OP.add)
            zt4 = big.tile([P, JC, 6], bf16, tag="zt4")
            nc.vector.tensor_tensor(out=zt4[:], in0=zt3[:, :, 0:6],
                                    in1=zt3[:, :, 6:12], op=OP.add)
            z_t = big.tile([P, JC], f32, tag="z")
            nc.vector.reduce_sum(out=z_t[:], in_=zt4[:], axis=AX.X)
            rz = big.tile([P, JC], f32, tag="rz")
            nc.vector.reciprocal(out=rz[:], in_=z_t[:])
            # replicate 1/Z into 4 contiguous bf16 lanes so the softmax
            # product below has a unit-stride 16-bit in1 -> DVE 2x mode
            rz4 = big.tile([P, JC, 4], bf16, tag="rz4")
            nc.vector.tensor_scalar(out=rz4[:], in0=bc(rz[:], 2, 4), scalar1=1.0,
                                    scalar2=None, op0=OP.mult)
            ep = big.tile([P, JC, M], bf16, tag="ep")
            ep_i = nc.vector.tensor_tensor(
                out=ep[:].rearrange("p j (g i) -> p j g i", i=4),
                in0=exp_t[:].rearrange("p j (g i) -> p j g i", i=4),
                in1=bc(rz4[:], 2, M // 4), op=OP.mult)
            for j in range(JC):
                nc.tensor.matmul(out=C_ps[:], lhsT=t_t[:, j, :], rhs=ep[:, j, :],
                                 start=(c == 0 and j == 0),
                                 stop=(c == NCHUNK - 1 and j == JC - 1))
            return exp_i, ep_i

        for c in range(NCHUNK):
            last_exp, last_ep = dice_chunk(c)

        def after_dice(inst):
            add_dep_helper(inst.ins, last_ep.ins, reason="tail after dice")
            return inst

        def after_exps(inst):
            add_dep_helper(inst.ins, last_exp.ins, reason="ln after exps")
            return inst

        # ---------- exp-side of occ / class / windows (same ACT table set) ----------
        e4 = sing.tile([P, J, K], bf16)
        nc.scalar.activation(out=e4[:], in_=occ_t[:], func=AF.Exp)
        expc = sing.tile([32, 5], f32)
        nc.scalar.activation(out=expc[:], in_=iel, func=AF.Exp)
        tv = sing.tile([M, WIN * WIN], f32)
        after_dice(nc.vector.tensor_copy(out=tv[:].rearrange("m (a b) -> m a b", a=WIN),
                                         in_=restride(twr[:], [[W, WIN], [1, WIN]])))
        lg = sing.tile([M, WIN * WIN], f32)
        after_dice(nc.vector.tensor_copy(out=lg[:].rearrange("m (a b) -> m a b", a=WIN),
                                         in_=restride(bwr[:], [[W, WIN], [1, WIN]])))
        expw = sing.tile([M, WIN * WIN], f32)
        nc.scalar.activation(out=expw[:], in_=lg[:], func=AF.Exp)
        s4 = sing.tile([P, J], f32)
        after_dice(nc.vector.reduce_sum(out=s4[:], in_=e4[:], axis=AX.X))

        # ---------- NLL prelude (f32 — the only term that needs precision) ----------
        d_ = sing.tile([M, 2], f32)
        after_dice(nc.vector.tensor_tensor(out=d_[:], in0=ptsr, in1=cenr, op=OP.subtract))
        r00 = sing.tile([M, 1], f32)
        nc.vector.reciprocal(out=r00[:], in_=cholr[:, 0:1])
        r11 = sing.tile([M, 1], f32)
        nc.vector.reciprocal(out=r11[:], in_=cholr[:, 3:4])
        z0 = sing.tile([M, 1], f32)
        nc.vector.tensor_tensor(out=z0[:], in0=d_[:, 0:1], in1=r00[:], op=OP.mult)
        t1 = sing.tile([M, 1], f32)
        nc.vector.tensor_tensor(out=t1[:], in0=cholr[:, 2:3], in1=z0[:], op=OP.mult)
        nc.vector.tensor_tensor(out=t1[:], in0=d_[:, 1:2], in1=t1[:], op=OP.subtract)
        z1 = sing.tile([M, 1], f32)
        nc.vector.tensor_tensor(out=z1[:], in0=t1[:], in1=r11[:], op=OP.mult)
        sq = sing.tile([M, 1], f32)
        nc.vector.tensor_tensor(out=sq[:], in0=z0[:], in1=z0[:], op=OP.mult)
        sq1 = sing.tile([M, 1], f32)
        nc.vector.tensor_tensor(out=sq1[:], in0=z1[:], in1=z1[:], op=OP.mult)
        nc.vector.tensor_tensor(out=sq[:], in0=sq[:], in1=sq1[:], op=OP.add)
        ldet = sing.tile([M, 1], f32)
        nc.vector.tensor_tensor(out=ldet[:], in0=cholr[:, 0:1], in1=cholr[:, 3:4], op=OP.mult)

        # ---------- Ln cluster (single ACT table switch, after all exps) ----------
        lse = sing.tile([P, J], f32)
        after_exps(nc.scalar.activation(out=lse[:], in_=s4[:], func=AF.Ln))
        sp = sing.tile([32, 5], f32)
        after_exps(nc.scalar.activation(out=sp[:], in_=expc[:], func=AF.Ln, bias=1.0))
        lnd = sing.tile([M, 1], f32)
        after_exps(nc.scalar.activation(out=lnd[:], in_=ldet[:], func=AF.Ln))
        spw = sing.tile([M, WIN * WIN], f32)
        after_exps(nc.scalar.activation(out=spw[:], in_=expw[:], func=AF.Ln, bias=1.0))

        # ---------- finishers ----------
        # occupancy CE
        d4 = sing.tile([P, J], f32)
        nc.gpsimd.tensor_tensor(out=d4[:], in0=lse[:], in1=xsel_t[:], op=OP.subtract)
        nc.vector.reduce_sum(out=stats[:, 4:5], in_=d4[:], axis=AX.X)
        # class loss (32 partitions, folded via the final ones-matmul)
        t9 = sing.tile([32, 5], f32)
        nc.vector.tensor_scalar(out=t9[:], in0=sp[:], scalar1=0.9, scalar2=None, op0=OP.mult)
        nc.vector.tensor_tensor(out=t9[:], in0=t9[:], in1=iel, op=OP.subtract)
        nc.vector.tensor_tensor(out=t9[:], in0=t9[:], in1=lab, op=OP.mult)
        nc.vector.reduce_sum(out=stats[0:32, 6:7], in_=t9[:], axis=AX.X)
        nc.vector.reduce_sum(out=stats[0:32, 5:6], in_=sp[:], axis=AX.X)
        # nll
        nc.vector.tensor_scalar(out=sq[:], in0=sq[:], scalar1=0.5,
                                scalar2=float(np.log(2.0 * np.pi)), op0=OP.mult, op1=OP.add)
        nc.vector.tensor_tensor(out=stats[0:M, 0:1], in0=sq[:], in1=lnd[:], op=OP.add)
        # window bce
        prw = sing.tile([M, WIN * WIN], f32)
        nc.gpsimd.tensor_tensor(out=prw[:], in0=lg[:], in1=tv[:], op=OP.mult)
        nc.gpsimd.tensor_tensor(out=prw[:], in0=spw[:], in1=prw[:], op=OP.subtract)
        valid49 = sing.tile([M, WIN * WIN], f32)
        nc.gpsimd.tensor_copy(out=valid49[:].rearrange("m (a b) -> m a b", a=WIN),
                              in_=bc(valid[:], 2, WIN))
        scr_w = sing.tile([M, WIN * WIN], f32)
        nc.gpsimd.tensor_tensor(out=scr_w[:], in0=prw[:], in1=valid49[:], op=OP.mult)
        nc.vector.reduce_sum(out=stats[0:M, 1:2], in_=scr_w[:], axis=AX.X)
        # dice: trace(C) and sum(C)
        Cs = sing.tile([M, M], f32)
        nc.vector.tensor_copy(out=Cs[:], in_=C_ps[:])
        scr_c = sing.tile([M, M], f32)
        nc.gpsimd.tensor_tensor(out=scr_c[:], in0=Cs[:], in1=ident[:], op=OP.mult)
        nc.vector.reduce_sum(out=stats[0:M, 2:3], in_=scr_c[:], axis=AX.X)
        nc.vector.reduce_sum(out=stats[0:M, 3:4], in_=Cs[:], axis=AX.X)

        # ---------- final cross-partition reduction ----------
        fin_ps = ps.tile([1, 8], f32)
        nc.tensor.matmul(out=fin_ps[:], lhsT=ones[:], rhs=stats[:], start=True, stop=True)
        nc.vector.tensor_copy(out=res[:, 0:8], in_=fin_ps[:])
        nc.sync.dma_start(out=partials.ap(), in_=res[:])

    nc.compile()
    return nc


def _get_nc():
    if "nc" not in _CACHE:
        _CACHE["nc"] = _build_nc()
    return _CACHE["nc"]


def make_in_maps(is_electron_logit, true_segmap, binary_mask_logits, portion_logits,
                 incidence_points, positions, chol, occupancy_logits, occupancy_true,
                 matched_q, matched_e):
    import ml_dtypes
    f = np.float32
    f8 = ml_dtypes.float8_e3m4
    f8e4 = ml_dtypes.float8_e4m3
    bf = ml_dtypes.bfloat16
    ident = np.eye(M, dtype=f)
    in_maps = []
    for b in range(B):
        me = np.asarray(matched_e[b])
        mq = np.asarray(matched_q[b])
        true_r = np.asarray(true_segmap[b])[:, :, me]          # [H, W, M]
        por_r = np.asarray(portion_logits[b])[:, :, mq]        # [H, W, M]
        bin_r = np.asarray(binary_mask_logits[b])[:, :, mq]    # [H, W, M]
        twin_b = np.ascontiguousarray(true_r.reshape(FULLPIX, M).T).astype(f8)
        bwin_b = np.ascontiguousarray(bin_r.reshape(FULLPIX, M).T).astype(f8)
        iel = np.asarray(is_electron_logit, dtype=f).reshape(B, Q)[b].reshape(32, 5)
        lab = np.zeros(Q, dtype=f)
        lab[mq] = 1.0
        lab = lab.reshape(32, 5)
        occ_b = np.asarray(occupancy_logits[b], dtype=f)
        occt_b = np.asarray(occupancy_true[b])
        xsel_b = np.take_along_axis(occ_b.reshape(FULLPIX, K),
                                    occt_b.reshape(FULLPIX, 1), axis=1)
        for h in range(2):
            sl = slice(h * HALF, (h + 1) * HALF)
            psl = slice(h * NPIX, (h + 1) * NPIX)
            smalls = np.zeros((P, NSM), dtype=f)
            smalls[0:M, 0:2] = np.asarray(incidence_points[b], dtype=f)[me]
            smalls[0:M, 2:4] = np.asarray(positions[b], dtype=f)[mq]
            smalls[0:M, 4:8] = np.asarray(chol[b], dtype=f).reshape(Q, 4)[mq]
            smalls[0:M, 8:15] = np.arange(WIN, dtype=f)[None, :]
            smalls[0:M, 15] = float(h * HALF)
            smalls[0:M, 16] = float(h * HALF + HALF - 1)
            smalls[0:M, 17] = np.arange(M, dtype=f) * FULLPIX
            smalls[0:32, 18:23] = iel
            smalls[0:32, 23:28] = lab
            in_maps.append(dict(
                por_sl=np.ascontiguousarray(por_r[sl]).reshape(NPIX, M).astype(f8),
                true_sl=np.ascontiguousarray(true_r[sl]).reshape(NPIX, M).astype(f8e4),
                occ_sl=np.ascontiguousarray(occ_b[sl]).reshape(P, J, K).astype(f8),
                xsel=np.ascontiguousarray(xsel_b[psl]).reshape(P, J).astype(bf),
                twin=twin_b, bwin=bwin_b,
                smalls=smalls, ident=ident,
            ))
    return in_maps


def combine(partials_list):
    s = np.stack([np.asarray(p, dtype=np.float64).reshape(8) for p in partials_list])
    # slots: 0=nll_sum 1=bce_sum 2=trace(C) 3=sum(C)=sum_true 4=occ_sum
    # 5=softplus_sum 6=matched(0.9*sp - x) sum
    class_loss = (NO_E * s[0::2, 5].sum() + s[0::2, 6].sum()) / (B * Q)
    nll_loss = s[0::2, 0].sum() / (B * M)
    bce_loss = s[:, 1].sum() / (B * M * WIN * WIN)
    occ_loss = s[:, 4].sum() / (B * H * W)
    dice = 0.0
    for b in range(B):
        num = 2.0 * (s[2 * b, 2] + s[2 * b + 1, 2])
        den = s[2 * b, 3] + s[2 * b + 1, 3] + H * W
        dice += 1.0 - (num + 1.0) / (den + 1.0)
    dice_loss = dice / B
    return np.float32(class_loss + bce_loss + dice_loss + nll_loss + occ_loss)


def kernel(**inputs):
    from concourse.bass_utils import run_bass_kernel_spmd
    nc = _get_nc()
    in_maps = make_in_maps(**{k: np.asarray(v) for k, v in inputs.items()})
    r = run_bass_kernel_spmd(nc, in_maps, list(range(8)))
    return combine([r.results[c]["partials"] for c in range(8)])
